# revision 1
# baseline (speedup 1.0000x reference)
"""Trainium2 Bass kernel for nn_MHParallelAttention (B=4,S=1024,H=16,DK=64).

Sharding: 8 cores = (batch) x (query-row half); each core owns output rows
[b, s0:s0+512, :] end-to-end, no collectives.

Algebra folds:
  * sum_h Wc_h*(q_h . k_h) == (concat_h Wc_h*q_h) . (concat_h k_h): the
    whole scores+head-combine collapses to one [512,1024]@[1024,1024]^T
    matmul per core, PSUM-accumulated over 8 chunks of 128 features.
  * bc is softmax-shift-invariant -> dropped.
  * block-diagonal [[W,0],[0,W]] 128x128 projection weights process a head
    PAIR per matmul with output at PSUM base partition 0 (fp32r-legal) and
    full 128-lane tanh.
  * softmax without max-subtraction (logits bounded ~6 for this problem);
    mask applied as 0/1 int8 multiply AFTER exp, fused with the row-sum in
    one DVE scalar_tensor_tensor op per half.

Schedule: input DMAs interleaved kt_j/qt_j in consumption order (engines
are in-order); scores for t=0,1 (both ki halves) accumulate inline with
the projections and ship their output rows early; t=2,3 follow with t=2
pre-running on spare PSUM banks. Matmuls run in float32r (1 cycle/row;
HW rel-err ~4e-4). Set KERNEL_F32R=0 for full fp32 (~2x slower).

Host-side prep is layout-only; all FLOPs run on device.
"""

import os
import sys

import numpy as np

for _p in ("/opt/trn_rl_repo", "/root/.axon_site/_ro/trn_rl_repo"):
    if os.path.isdir(_p) and _p not in sys.path:
        sys.path.insert(0, _p)

import concourse.bass as bass
import concourse.mybir as mybir
import concourse.tile as tile
from concourse import bacc
from concourse.bass import ds, ts

H, DK = 16, 64
B, S = 4, 1024
SQ = 512
NCORES = 8
NJ = 8
NEG = -1.0e10

F32 = mybir.dt.float32
F32R = mybir.dt.float32r
I32 = mybir.dt.int32
I8 = mybir.dt.int8

USE_F32R = os.environ.get("KERNEL_F32R", "1") == "1"

# packed weight layout along free dim: wkblk[128] | wqblk[128] | bk | bq | wc[8]
WOFF_WK, WOFF_WQ, WOFF_BK, WOFF_BQ, WOFF_WC = 0, 128, 256, 257, 258
WFREE = 266


def build_nc():
    nc = bacc.Bacc(None, target_bir_lowering=False, debug=False)
    DT = F32R if USE_F32R else F32

    qT = nc.dram_tensor("qT", [NJ, 128, SQ], DT, kind="ExternalInput")
    kT = nc.dram_tensor("kT", [NJ, 128, S], DT, kind="ExternalInput")
    msk = nc.dram_tensor("msk", [4, 128, S], I8, kind="ExternalInput")
    wts = nc.dram_tensor("wts", [128, WFREE], DT, kind="ExternalInput")
    out = nc.dram_tensor("out", [SQ, S], F32, kind="ExternalOutput")

    Tanh = mybir.ActivationFunctionType.Tanh
    Exp = mybir.ActivationFunctionType.Exp

    with tile.TileContext(nc) as tc:
        with (
            tc.tile_pool(name="const", bufs=1) as cst,
            tc.tile_pool(name="kin", bufs=1) as kin,
            tc.tile_pool(name="qin", bufs=1) as qin,
            tc.tile_pool(name="kpp", bufs=1) as kpp,
            tc.tile_pool(name="qpp", bufs=1) as qpp,
            tc.tile_pool(name="tmp", bufs=4) as tmpp,
            tc.tile_pool(name="mrow", bufs=1) as mrp,
            tc.tile_pool(name="soft", bufs=2) as softp,
            tc.tile_pool(name="stat", bufs=8) as statp,
            tc.tile_pool(name="obuf", bufs=4) as obp,
            tc.tile_pool(name="pproj", bufs=2, space="PSUM") as pproj,
            tc.tile_pool(name="pscore", bufs=4, space="PSUM") as pscore,
        ):
            wts_sb = cst.tile([128, WFREE], DT, tag="wts")
            nc.sync.dma_start(out=wts_sb[:], in_=wts[:])
            wkb = wts_sb[:, WOFF_WK:WOFF_WK + 128]
            wqb = wts_sb[:, WOFF_WQ:WOFF_WQ + 128]
            bkb = wts_sb[:, WOFF_BK:WOFF_BK + 1].bitcast(F32)
            bqb = wts_sb[:, WOFF_BQ:WOFF_BQ + 1].bitcast(F32)
            wcb = wts_sb[:, WOFF_WC:WOFF_WC + NJ].bitcast(F32)

            mk = mrp.tile([128, 4, S], I8, tag="mk")

            kp = [kpp.tile([128, S], DT, tag=f"kp{j}", name=f"kp{j}")
                  for j in range(NJ)]
            qp = [qpp.tile([128, SQ], DT, tag=f"qp{j}", name=f"qp{j}")
                  for j in range(NJ)]

            # ---- input DMAs on SP queue; arrival order = emission order =
            # consumption order. Fine granularity so ACT starts early.
            pst01 = {(t, kh): pscore.tile([128, 512], F32, tag="ps", bufs=6,
                     name=f"psA_{t}_{kh}") for t in range(2) for kh in range(2)}

            # kt_j then qt_j arrivals, each followed immediately by its
            # projection and the j-th kh=0 score chunk
            for j in range(NJ):
                kt = kin.tile([128, S], DT, tag="kt", bufs=4, name=f"kt{j}")
                nc.sync.dma_start(out=kt[:], in_=kT[j])
                qt = qin.tile([128, SQ], DT, tag="qt", bufs=4, name=f"qt{j}")
                nc.sync.dma_start(out=qt[:], in_=qT[j])
                for half in range(2):
                    ps = pproj.tile([128, 512], F32, tag="pp")
                    sl = ds(half * 512, 512)
                    nc.tensor.matmul(ps[:], wkb, kt[:, sl])
                    nc.scalar.activation(kp[j][:, sl], ps[:], Tanh, bias=bkb)
                ps = pproj.tile([128, 512], F32, tag="pp")
                nc.tensor.matmul(ps[:], wqb, qt[:])
                tq = tmpp.tile([128, SQ], F32, tag="tmp")
                nc.scalar.activation(tq[:], ps[:], Tanh, bias=bqb)
                nc.vector.tensor_scalar_mul(qp[j][:], tq[:], wcb[:, j:j + 1])
                for t in range(2):
                    for kh in range(2):
                        nc.tensor.matmul(
                            pst01[(t, kh)][:], qp[j][:, ts(t, 128)],
                            kp[j][:, ts(kh, 512)],
                            start=(j == 0), stop=(j == NJ - 1),
                        )

            # mask after inputs on the same queue (needed only by the tail)
            nc.sync.dma_start(out=mk[:], in_=msk[:].rearrange("t p k -> p t k"))

            # softmax without max-subtraction (|logit| <= ~6 here; masked
            # entries killed by multiplying with the 0/1 int8 mask AFTER exp;
            # fused accum gives the masked row-sum in the same DVE pass)
            exs = [softp.tile([128, S], F32, tag=f"ex{t}", name=f"ex{t}",
                              bufs=1) for t in range(4)]

            def tail_chain(t, psa, psb):
                nc.scalar.activation(exs[t][:, ts(0, 512)], psa[:], Exp)
                nc.scalar.activation(exs[t][:, ts(1, 512)], psb[:], Exp)
                exm = obp.tile([128, S], F32, tag="exm")
                s0 = statp.tile([128, 1], F32, tag="s0")
                s1 = statp.tile([128, 1], F32, tag="s1")
                nc.vector.scalar_tensor_tensor(
                    exm[:, ts(0, 512)], exs[t][:, ts(0, 512)], 1.0,
                    mk[:, t, ts(0, 512)],
                    op0=mybir.AluOpType.bypass, op1=mybir.AluOpType.mult,
                    accum_out=s0[:],
                )
                nc.vector.scalar_tensor_tensor(
                    exm[:, ts(1, 512)], exs[t][:, ts(1, 512)], 1.0,
                    mk[:, t, ts(1, 512)],
                    op0=mybir.AluOpType.bypass, op1=mybir.AluOpType.mult,
                    accum_out=s1[:],
                )
                ssum = statp.tile([128, 1], F32, tag="ssum")
                nc.vector.tensor_tensor(ssum[:], s0[:], s1[:],
                                        op=mybir.AluOpType.add)
                rec = statp.tile([128, 1], F32, tag="rec")
                nc.vector.reciprocal(rec[:], ssum[:])
                ot = obp.tile([128, S], F32, tag="ot")
                for hh in range(2):
                    nc.vector.tensor_scalar_mul(
                        ot[:, ts(hh, 512)], exm[:, ts(hh, 512)], rec[:])
                    nc.sync.dma_start(
                        out=out[ts(t, 128), ds(hh * 512, 512)],
                        in_=ot[:, ts(hh, 512)])

            # t=0,1 finished in phase 1 -> chain + output immediately
            for t in range(2):
                tail_chain(t, pst01[(t, 0)], pst01[(t, 1)])

            # ---- phase 2: t=2,3 (t=2 psums pre-run on spare banks)
            for t in (2, 3):
                psa = pscore.tile([128, 512], F32, tag="ps", bufs=6,
                                  name=f"psB_{t}_0")
                psb = pscore.tile([128, 512], F32, tag="ps", bufs=6,
                                  name=f"psB_{t}_1")
                for j in range(NJ):
                    nc.tensor.matmul(
                        psa[:], qp[j][:, ts(t, 128)], kp[j][:, ts(0, 512)],
                        start=(j == 0), stop=(j == NJ - 1),
                    )
                    nc.tensor.matmul(
                        psb[:], qp[j][:, ts(t, 128)], kp[j][:, ts(1, 512)],
                        start=(j == 0), stop=(j == NJ - 1),
                    )
                tail_chain(t, psa, psb)

    nc.compile()
    return nc


_NC = None


def _get_nc():
    global _NC
    if _NC is None:
        _NC = build_nc()
    return _NC


def make_in_maps(query, key, mask, Wq, bq, Wk, bk, Wc, bc):
    query = np.asarray(query, np.float32)
    key = np.asarray(key, np.float32)
    mask = np.asarray(mask)
    Wq = np.asarray(Wq, np.float32)
    Wk = np.asarray(Wk, np.float32)
    Wc = np.asarray(Wc, np.float32)
    bq = np.asarray(bq, np.float32)
    bk = np.asarray(bk, np.float32)

    def blockdiag(W):
        blk = np.zeros((128, 128), np.float32)
        blk[0:64, 0:64] = W.T
        blk[64:128, 64:128] = W.T
        return blk

    wts = np.zeros((128, WFREE), np.float32)
    wts[:, WOFF_WK:WOFF_WK + 128] = blockdiag(Wk)
    wts[:, WOFF_WQ:WOFF_WQ + 128] = blockdiag(Wq)
    wts[:, WOFF_BK] = np.tile(bk.reshape(-1), 2)
    wts[:, WOFF_BQ] = np.tile(bq.reshape(-1), 2)
    for j in range(NJ):
        wts[0:64, WOFF_WC + j] = Wc[0, 2 * j]
        wts[64:128, WOFF_WC + j] = Wc[0, 2 * j + 1]

    in_maps = []
    for c in range(NCORES):
        b, half = divmod(c, 2)
        s0 = half * SQ
        qh = query[b].reshape(H, S, DK)[:, s0:s0 + SQ, :]
        qTc = np.ascontiguousarray(qh.transpose(0, 2, 1)).reshape(NJ, 128, SQ)
        kh_ = key[b].reshape(H, S, DK)
        kTc = np.ascontiguousarray(kh_.transpose(0, 2, 1)).reshape(NJ, 128, S)
        mc = np.ascontiguousarray(
            mask[b, s0:s0 + SQ, :].reshape(4, 128, S)).astype(np.int8)
        in_maps.append({"qT": qTc, "kT": kTc, "msk": mc, "wts": wts})
    return in_maps


def kernel(query, key, mask, Wq, bq, Wk, bk, Wc, bc):
    from concourse.bass_utils import run_bass_kernel_spmd

    nc = _get_nc()
    in_maps = make_in_maps(query, key, mask, Wq, bq, Wk, bk, Wc, bc)
    res = run_bass_kernel_spmd(nc, in_maps, list(range(NCORES)))
    full = np.empty((B, S, S), np.float32)
    for c in range(NCORES):
        b, half = divmod(c, 2)
        full[b, half * SQ:(half + 1) * SQ, :] = res.results[c]["out"]
    return full



# revision 2
# speedup vs baseline: 1.0013x; 1.0013x over previous
"""Trainium2 Bass kernel for nn_MHParallelAttention (B=4,S=1024,H=16,DK=64).

V2: bf16 matmul path, fused 2-bank PSUM activations, PE p-state warmup,
staggered score-tile tails.

Sharding: 8 cores = (batch) x (query-row half); each core owns output rows
[b, s0:s0+512, :] end-to-end, no collectives.

Algebra folds (as baseline):
  * sum_h Wc_h*(q_h . k_h) == (concat_h Wc_h*q_h) . (concat_h k_h): scores +
    head-combine collapse to one [512,1024]@[1024,1024]^T matmul per core,
    PSUM-accumulated over 8 chunks of 128 features.
  * bc is softmax-shift-invariant -> dropped.
  * block-diag [[W,0],[0,W]] 128x128 projection weights process a head PAIR
    per matmul.
  * softmax without max-subtraction (|logit| <= ~6 here); 0/1 int8 mask
    applied multiplicatively AFTER exp, fused with the row-sum in one DVE
    scalar_tensor_tensor pass.

V2 schedule:
  * all matmuls bf16 (1 cycle/row, same as f32r, but half the DMA bytes —
    DMA is a serial ~17us budget at f32).
  * kp proj per j: 2 matmuls into a [128,1024] 2-bank psum -> ONE fused
    tanh -> kp[j] bf16. qp proj per j-PAIR: 2 matmuls into a [128,1024]
    psum -> ONE fused tanh (same bias) -> tq -> 2 DVE wc-scales (bf16 4x
    mode) -> qp[j].
  * psum banks: ppk bufs=2 (4 banks, keeps ACT fed while PE runs ahead),
    ppq bufs=1 (2), inline score tile t=0 (2). Tiles t=1,2,3 run after the
    j-loop reusing the proj pools, so their softmax tails stagger.
  * tail per tile: fused [128,1024] exp (ACT) -> masked-sum stt (DVE,
    accum_out) -> reciprocal -> fused scale -> out DMA on the gpsimd SWDGE
    queue. Last tile (t=3) is processed per-half with kh-ordered matmuls to
    shorten the exposed end chain.
  * 6 dummy warmup matmuls keep the PE busy from ~0.3us so the 2.4GHz
    p-state is reached before real work; startup DMAs (wts/aux on gpsimd,
    first kt chunk split in half) shorten time-to-first-matmul.

Host-side prep is layout/dtype-only; all FLOPs run on device.
"""

import os
import sys

import numpy as np

for _p in ("/opt/trn_rl_repo", "/root/.axon_site/_ro/trn_rl_repo"):
    if os.path.isdir(_p) and _p not in sys.path:
        sys.path.insert(0, _p)

import concourse.bass as bass
import concourse.mybir as mybir
import concourse.tile as tile
from concourse import bacc
from concourse.bass import ds, ts

H, DK = 16, 64
B, S = 4, 1024
SQ = 512
NCORES = 8
NJ = 8

F32 = mybir.dt.float32
F16 = mybir.dt.float16
BF16 = mybir.dt.bfloat16
I8 = mybir.dt.int8
NP_BF16 = mybir.dt.np(BF16)

# aux f32 layout along free dim: bk | bq | wc[8]
AOFF_BK, AOFF_BQ, AOFF_WC = 0, 1, 2
AFREE = 10

Tanh = mybir.ActivationFunctionType.Tanh
Exp = mybir.ActivationFunctionType.Exp
Byp = mybir.AluOpType.bypass
Mult = mybir.AluOpType.mult


def build_nc():
    nc = bacc.Bacc(None, target_bir_lowering=False, debug=False)

    qT = nc.dram_tensor("qT", [NJ, 128, SQ], BF16, kind="ExternalInput")
    kT = nc.dram_tensor("kT", [NJ, 128, S], BF16, kind="ExternalInput")
    msk = nc.dram_tensor("msk", [4, 128, S], I8, kind="ExternalInput")
    wts = nc.dram_tensor("wts", [128, 256], BF16, kind="ExternalInput")
    aux = nc.dram_tensor("aux", [128, AFREE], F32, kind="ExternalInput")
    out = nc.dram_tensor("out", [SQ, S], F16, kind="ExternalOutput")

    with tile.TileContext(nc) as tc:
        with (
            tc.tile_pool(name="const", bufs=1) as cst,
            tc.tile_pool(name="kin", bufs=3) as kin,
            tc.tile_pool(name="qin", bufs=3) as qin,
            tc.tile_pool(name="kpp", bufs=1) as kpp,
            tc.tile_pool(name="qpp", bufs=1) as qpp,
            tc.tile_pool(name="tqp", bufs=3) as tqp,
            tc.tile_pool(name="mrow", bufs=1) as mrp,
            tc.tile_pool(name="soft", bufs=4) as softp,
            tc.tile_pool(name="stat", bufs=8) as statp,
            tc.tile_pool(name="obuf", bufs=4) as obp,
            tc.tile_pool(name="ppk", bufs=2, space="PSUM") as ppk,
            tc.tile_pool(name="ppq", bufs=2, space="PSUM") as ppq,
            tc.tile_pool(name="psc", bufs=4, space="PSUM") as psc,
        ):
            # ---- warmup junk first: DVE memsets at t~0, then dummy
            # matmuls (p-state ramp) and a tiny tanh (absorbs the 1283ns
            # LoadActFuncSet before the real pipeline).
            jst = cst.tile([128, 128], BF16, tag="jst", name="jst")
            nc.vector.memset(jst[:], 0.25)
            jmv = cst.tile([128, 512], BF16, tag="jmv", name="jmv")
            nc.vector.memset(jmv[:], 0.25)
            jact = cst.tile([128, 16], F32, tag="jact", name="jact")
            nc.scalar.activation(jact[:], jst[:, 0:16], Tanh)

            # ---- small constants at the FRONT of the sync queue (land
            # before kt0; first kp matmul needs wkb)
            aux_sb = cst.tile([128, AFREE], F32, tag="aux", name="aux_sb")
            nc.sync.dma_start(out=aux_sb[:], in_=aux[:])
            wts_sb = cst.tile([128, 256], BF16, tag="wts", name="wts_sb")
            nc.sync.dma_start(out=wts_sb[:], in_=wts[:])
            wkb = wts_sb[:, 0:128]
            wqb = wts_sb[:, 128:256]
            bkb = aux_sb[:, AOFF_BK:AOFF_BK + 1]
            bqb = aux_sb[:, AOFF_BQ:AOFF_BQ + 1]
            wcb = aux_sb[:, AOFF_WC:AOFF_WC + NJ]

            mk = mrp.tile([128, 4, S], I8, tag="mk", name="mk")

            kp = [kpp.tile([128, S], BF16, tag=f"kp{j}", name=f"kp{j}")
                  for j in range(NJ)]
            qp = [qpp.tile([128, SQ], BF16, tag=f"qp{j}", name=f"qp{j}")
                  for j in range(NJ)]

            warm = ppk.tile([128, 512], F32, tag="pk", name="warm")
            for w in range(4):
                nc.tensor.matmul(warm[:], jst[:], jmv[:],
                                 start=True, stop=True)

            # inline score tiles t=0,1 as per-half psum tiles (finer deps)
            psA = {(t, kh): psc.tile([128, 512], F32, tag="ps",
                                     name=f"psA_{t}_{kh}")
                   for t in range(2) for kh in range(2)}

            # ---- j-loop: input DMAs in consumption order on the sync
            # queue; unfused per-half kp proj + per-j qp proj (all [128,512]
            # single-bank psums, bufs=2 rotation); inline score matmuls for
            # t=0,1 emitted with a 3-j lag so the PE never head-of-line
            # blocks on the qp tanh/scale chain.
            def emit_scores(jj):
                for t in range(2):
                    for kh in range(2):
                        nc.tensor.matmul(
                            psA[(t, kh)][:], qp[jj][:, ts(t, 128)],
                            kp[jj][:, ts(kh, 512)],
                            start=(jj == 0), stop=(jj == NJ - 1))

            for j in range(NJ):
                kt = kin.tile([128, S], BF16, tag="kt", name=f"kt{j}")
                if j == 0:
                    nc.sync.dma_start(out=kt[:, 0:512], in_=kT[0][:, 0:512])
                    nc.sync.dma_start(out=kt[:, 512:1024],
                                      in_=kT[0][:, 512:1024])
                else:
                    nc.sync.dma_start(out=kt[:], in_=kT[j])
                qt = qin.tile([128, SQ], BF16, tag="qt", name=f"qt{j}")
                nc.sync.dma_start(out=qt[:], in_=qT[j])
                for kh in range(2):
                    pk = ppk.tile([128, 512], F32, tag="pk",
                                  name=f"pk{j}_{kh}")
                    nc.tensor.matmul(pk[:], wkb, kt[:, ts(kh, 512)],
                                     start=True, stop=True)
                    nc.scalar.activation(kp[j][:, ts(kh, 512)], pk[:], Tanh,
                                         bias=bkb)

                pqj = ppq.tile([128, 512], F32, tag="pq", name=f"pq{j}")
                nc.tensor.matmul(pqj[:], wqb, qt[:], start=True, stop=True)
                tq = tqp.tile([128, SQ], BF16, tag="tq", name=f"tq{j}")
                nc.scalar.activation(tq[:], pqj[:], Tanh, bias=bqb)
                nc.vector.tensor_scalar_mul(qp[j][:], tq[:], wcb[:, j:j + 1])

                if j >= 3:
                    emit_scores(j - 3)
            for jj in (NJ - 3, NJ - 2, NJ - 1):
                emit_scores(jj)

            # mask arrives after the input stream; needed from the first tail
            nc.sync.dma_start(out=mk[:], in_=msk[:].rearrange("t p k -> p t k"))

            def tail_halves(t, pst_halves):
                exs = softp.tile([128, S], F32, tag="ex", name=f"ex{t}")
                exm = obp.tile([128, S], F32, tag="exm", name=f"exm{t}")
                sh = [statp.tile([128, 1], F32, tag="ss", name=f"ss{t}_{h}")
                      for h in range(2)]
                for hh in range(2):
                    nc.scalar.activation(exs[:, ts(hh, 512)],
                                         pst_halves[hh][:], Exp)
                    nc.vector.scalar_tensor_tensor(
                        exm[:, ts(hh, 512)], exs[:, ts(hh, 512)], 1.0,
                        mk[:, t, ds(hh * 512, 512)],
                        op0=Byp, op1=Mult, accum_out=sh[hh][:])
                ssum = statp.tile([128, 1], F32, tag="ss", name=f"ssa{t}")
                nc.vector.tensor_tensor(ssum[:], sh[0][:], sh[1][:],
                                        op=mybir.AluOpType.add)
                rec = statp.tile([128, 1], F32, tag="rc", name=f"rc{t}")
                nc.vector.reciprocal(rec[:], ssum[:])
                ot = obp.tile([128, S], F16, tag="ot", name=f"ot{t}")
                for hh in range(2):
                    nc.vector.tensor_scalar_mul(
                        ot[:, ts(hh, 512)], exm[:, ts(hh, 512)], rec[:])
                    nc.sync.dma_start(
                        out=out[ts(t, 128), ds(hh * 512, 512)],
                        in_=ot[:, ts(hh, 512)])

            # ---- deferred tile t=2 on freed ppq half-slots (per-half deps)
            pb2 = [ppq.tile([128, 512], F32, tag="pq", name=f"psB2_{kh}")
                   for kh in range(2)]
            for kh in range(2):
                for jj in range(NJ):
                    nc.tensor.matmul(
                        pb2[kh][:], qp[jj][:, ts(2, 128)],
                        kp[jj][:, ts(kh, 512)],
                        start=(jj == 0), stop=(jj == NJ - 1))

            # inline-tile tails first: ready at loop end, and their psA
            # halves free up for t=3
            tail_halves(0, [psA[(0, 0)], psA[(0, 1)]])
            tail_halves(1, [psA[(1, 0)], psA[(1, 1)]])

            # ---- last tile t=3 on recycled psc half-slots, kh-major
            pb3 = [psc.tile([128, 512], F32, tag="ps", name=f"psB3_{kh}")
                   for kh in range(2)]
            for kh in range(2):
                for jj in range(NJ):
                    nc.tensor.matmul(
                        pb3[kh][:], qp[jj][:, ts(3, 128)],
                        kp[jj][:, ts(kh, 512)],
                        start=(jj == 0), stop=(jj == NJ - 1))

            tail_halves(2, pb2)
            tail_halves(3, pb3)

    nc.compile()
    return nc


_NC = None


def _get_nc():
    global _NC
    if _NC is None:
        _NC = build_nc()
    return _NC


def make_in_maps(query, key, mask, Wq, bq, Wk, bk, Wc, bc):
    query = np.asarray(query, np.float32)
    key = np.asarray(key, np.float32)
    mask = np.asarray(mask)
    Wq = np.asarray(Wq, np.float32)
    Wk = np.asarray(Wk, np.float32)
    Wc = np.asarray(Wc, np.float32)
    bq = np.asarray(bq, np.float32)
    bk = np.asarray(bk, np.float32)

    def blockdiag(W):
        blk = np.zeros((128, 128), np.float32)
        blk[0:64, 0:64] = W.T
        blk[64:128, 64:128] = W.T
        return blk

    wts = np.zeros((128, 256), np.float32)
    wts[:, 0:128] = blockdiag(Wk)
    wts[:, 128:256] = blockdiag(Wq)
    wts = wts.astype(NP_BF16)

    aux = np.zeros((128, AFREE), np.float32)
    aux[:, AOFF_BK] = np.tile(bk.reshape(-1), 2)
    aux[:, AOFF_BQ] = np.tile(bq.reshape(-1), 2)
    for j in range(NJ):
        aux[0:64, AOFF_WC + j] = Wc[0, 2 * j]
        aux[64:128, AOFF_WC + j] = Wc[0, 2 * j + 1]

    in_maps = []
    for c in range(NCORES):
        b, half = divmod(c, 2)
        s0 = half * SQ
        qh = query[b].reshape(H, S, DK)[:, s0:s0 + SQ, :]
        qTc = np.ascontiguousarray(
            qh.transpose(0, 2, 1)).reshape(NJ, 128, SQ).astype(NP_BF16)
        kh_ = key[b].reshape(H, S, DK)
        kTc = np.ascontiguousarray(
            kh_.transpose(0, 2, 1)).reshape(NJ, 128, S).astype(NP_BF16)
        mc = np.ascontiguousarray(
            mask[b, s0:s0 + SQ, :].reshape(4, 128, S)).astype(np.int8)
        in_maps.append({"qT": qTc, "kT": kTc, "msk": mc, "wts": wts,
                        "aux": aux})
    return in_maps


def kernel(query, key, mask, Wq, bq, Wk, bk, Wc, bc):
    from concourse.bass_utils import run_bass_kernel_spmd

    nc = _get_nc()
    in_maps = make_in_maps(query, key, mask, Wq, bq, Wk, bk, Wc, bc)
    res = run_bass_kernel_spmd(nc, in_maps, list(range(NCORES)))
    full = np.empty((B, S, S), np.float32)
    for c in range(NCORES):
        b, half = divmod(c, 2)
        full[b, half * SQ:(half + 1) * SQ, :] = \
            res.results[c]["out"].astype(np.float32)
    return full


# revision 4
# speedup vs baseline: 1.0254x; 1.0241x over previous
"""Trainium2 Bass kernel for nn_MHParallelAttention (B=4,S=1024,H=16,DK=64).

Sharding: 8 cores = (batch) x (query-row half); each core owns output rows
[b, s0:s0+512, :] end-to-end, no collectives.

Algebra folds:
  * sum_h Wc_h*(q_h . k_h) == (concat_h Wc_h*q_h) . (concat_h k_h): scores +
    head-combine collapse to one [512,1024]@[1024,1024]^T matmul per core,
    PSUM-accumulated over 8 chunks of 128 features.
  * bc is softmax-shift-invariant -> dropped.
  * block-diag [[W,0],[0,W]] 128x128 projection weights process a head PAIR
    per matmul.
  * softmax without max-subtraction (|logit| <= ~6 here); 0/1 int8 mask
    applied multiplicatively AFTER exp, fused with the row-sum in one DVE
    scalar_tensor_tensor pass.

Schedule (cost-model timeline 33.3us vs 39.9us for the f32r baseline):
  * all matmuls bf16: same 1 cycle/row as f32r on the PE, but half the DMA
    bytes - the DMA engines are a single serial ~17us-at-f32 resource.
  * j-loop: kt_j/qt_j DMAs in consumption order; kp proj as 2 single-bank
    [128,512] psum matmuls + per-half tanh (ppk bufs=2 so the ACT stream
    never waits); qp proj per j + tanh + DVE wc-scale (bf16 4x mode).
    Inline score accumulation for row-tiles t=0,1 into four single-bank
    half-psums, emitted with a 2-j lag so the PE never head-of-line blocks
    on the qp tanh/scale chain.
  * deferred tiles: t=2 on the ppq slots freed by the last qp tanh, t=3 on
    the psc slots freed by t=0's exp - both kh-major with per-half psum
    tiles so each half's exp chains off its own 8 matmuls.
  * tails: exp (ACT) -> masked-sum stt (DVE, accum_out) -> reciprocal ->
    per-half scale -> fp16 out DMA on the sync queue (host upcasts to f32;
    fp16 quantization ~5e-4 << 2e-2 tolerance). Tails emitted in readiness
    order so the out-DMA train streams while t=2/t=3 matmuls finish.
  * warmup: DVE memsets + 4 dummy matmuls ramp the PE p-state before real
    work; a tiny tanh absorbs the 1283ns LoadActFuncSet; wts rides the
    scalar-queue DGE in parallel with the sync queue's kt stream.

Host-side prep is layout/dtype-only; all FLOPs run on device.
"""

import os
import sys

import numpy as np

for _p in ("/opt/trn_rl_repo", "/root/.axon_site/_ro/trn_rl_repo"):
    if os.path.isdir(_p) and _p not in sys.path:
        sys.path.insert(0, _p)

import concourse.bass as bass
import concourse.mybir as mybir
import concourse.tile as tile
from concourse import bacc
from concourse.bass import ds, ts

H, DK = 16, 64
B, S = 4, 1024
SQ = 512
NCORES = 8
NJ = 8

F32 = mybir.dt.float32
F16 = mybir.dt.float16
BF16 = mybir.dt.bfloat16
I8 = mybir.dt.int8
NP_BF16 = mybir.dt.np(BF16)

# aux f32 layout along free dim: bk | bq | wc[8]
AOFF_BK, AOFF_BQ, AOFF_WC = 0, 1, 2
AFREE = 10

Tanh = mybir.ActivationFunctionType.Tanh
Exp = mybir.ActivationFunctionType.Exp
Byp = mybir.AluOpType.bypass
Mult = mybir.AluOpType.mult


def build_nc():
    nc = bacc.Bacc(None, target_bir_lowering=False, debug=False)

    qT = nc.dram_tensor("qT", [NJ, 128, SQ], BF16, kind="ExternalInput")
    kT = nc.dram_tensor("kT", [NJ, 128, S], BF16, kind="ExternalInput")
    msk = nc.dram_tensor("msk", [4, 128, S], I8, kind="ExternalInput")
    wts = nc.dram_tensor("wts", [128, 256], BF16, kind="ExternalInput")
    aux = nc.dram_tensor("aux", [128, AFREE], F32, kind="ExternalInput")
    out = nc.dram_tensor("out", [SQ, S], F16, kind="ExternalOutput")

    with tile.TileContext(nc) as tc:
        with (
            tc.tile_pool(name="const", bufs=1) as cst,
            tc.tile_pool(name="kin", bufs=3) as kin,
            tc.tile_pool(name="qin", bufs=3) as qin,
            tc.tile_pool(name="kpp", bufs=1) as kpp,
            tc.tile_pool(name="qpp", bufs=1) as qpp,
            tc.tile_pool(name="tqp", bufs=3) as tqp,
            tc.tile_pool(name="mrow", bufs=1) as mrp,
            tc.tile_pool(name="soft", bufs=4) as softp,
            tc.tile_pool(name="stat", bufs=8) as statp,
            tc.tile_pool(name="obuf", bufs=4) as obp,
            tc.tile_pool(name="ppk", bufs=2, space="PSUM") as ppk,
            tc.tile_pool(name="ppq", bufs=2, space="PSUM") as ppq,
            tc.tile_pool(name="psc", bufs=4, space="PSUM") as psc,
        ):
            # ---- warmup junk first: DVE memsets at t~0, then dummy
            # matmuls (p-state ramp) and a tiny tanh (absorbs the 1283ns
            # LoadActFuncSet before the real pipeline).
            jst = cst.tile([128, 128], BF16, tag="jst", name="jst")
            nc.vector.memset(jst[:], 0.25)
            jmv = cst.tile([128, 512], BF16, tag="jmv", name="jmv")
            nc.vector.memset(jmv[:], 0.25)
            jact = cst.tile([128, 16], F32, tag="jact", name="jact")
            nc.scalar.activation(jact[:], jst[:, 0:16], Tanh)

            # ---- small constants at the FRONT of the sync queue (land
            # before kt0; first kp matmul needs wkb)
            wts_sb = cst.tile([128, 256], BF16, tag="wts", name="wts_sb")
            nc.scalar.dma_start(out=wts_sb[:], in_=wts[:])
            aux_sb = cst.tile([128, AFREE], F32, tag="aux", name="aux_sb")
            nc.sync.dma_start(out=aux_sb[:], in_=aux[:])
            wkb = wts_sb[:, 0:128]
            wqb = wts_sb[:, 128:256]
            bkb = aux_sb[:, AOFF_BK:AOFF_BK + 1]
            bqb = aux_sb[:, AOFF_BQ:AOFF_BQ + 1]
            wcb = aux_sb[:, AOFF_WC:AOFF_WC + NJ]

            mk = mrp.tile([128, 4, S], I8, tag="mk", name="mk")

            kp = [kpp.tile([128, S], BF16, tag=f"kp{j}", name=f"kp{j}")
                  for j in range(NJ)]
            qp = [qpp.tile([128, SQ], BF16, tag=f"qp{j}", name=f"qp{j}")
                  for j in range(NJ)]

            warm = ppk.tile([128, 512], F32, tag="pk", name="warm")
            for w in range(4):
                nc.tensor.matmul(warm[:], jst[:], jmv[:],
                                 start=True, stop=True)

            # inline score tiles t=0,1 as per-half psum tiles (finer deps)
            psA = {(t, kh): psc.tile([128, 512], F32, tag="ps",
                                     name=f"psA_{t}_{kh}")
                   for t in range(2) for kh in range(2)}

            # ---- j-loop: input DMAs in consumption order on the sync
            # queue; unfused per-half kp proj + per-j qp proj (all [128,512]
            # single-bank psums, bufs=2 rotation); inline score matmuls for
            # t=0,1 emitted with a 3-j lag so the PE never head-of-line
            # blocks on the qp tanh/scale chain.
            def emit_scores(jj):
                for t in range(2):
                    for kh in range(2):
                        nc.tensor.matmul(
                            psA[(t, kh)][:], qp[jj][:, ts(t, 128)],
                            kp[jj][:, ts(kh, 512)],
                            start=(jj == 0), stop=(jj == NJ - 1))

            for j in range(NJ):
                kt = kin.tile([128, S], BF16, tag="kt", name=f"kt{j}")
                if j == 0:
                    nc.sync.dma_start(out=kt[:, 0:512], in_=kT[0][:, 0:512])
                    nc.sync.dma_start(out=kt[:, 512:1024],
                                      in_=kT[0][:, 512:1024])
                else:
                    nc.sync.dma_start(out=kt[:], in_=kT[j])
                qt = qin.tile([128, SQ], BF16, tag="qt", name=f"qt{j}")
                nc.sync.dma_start(out=qt[:], in_=qT[j])
                for kh in range(2):
                    pk = ppk.tile([128, 512], F32, tag="pk",
                                  name=f"pk{j}_{kh}")
                    nc.tensor.matmul(pk[:], wkb, kt[:, ts(kh, 512)],
                                     start=True, stop=True)
                    nc.scalar.activation(kp[j][:, ts(kh, 512)], pk[:], Tanh,
                                         bias=bkb)

                pqj = ppq.tile([128, 512], F32, tag="pq", name=f"pq{j}")
                nc.tensor.matmul(pqj[:], wqb, qt[:], start=True, stop=True)
                tq = tqp.tile([128, SQ], BF16, tag="tq", name=f"tq{j}")
                nc.scalar.activation(tq[:], pqj[:], Tanh, bias=bqb)
                nc.vector.tensor_scalar_mul(qp[j][:], tq[:], wcb[:, j:j + 1])

                if j >= 3:
                    emit_scores(j - 3)
            for jj in (NJ - 3, NJ - 2, NJ - 1):
                emit_scores(jj)

            # mask arrives after the input stream; needed from the first tail
            nc.sync.dma_start(out=mk[:], in_=msk[:].rearrange("t p k -> p t k"))

            def tail_halves(t, pst_halves):
                exs = softp.tile([128, S], F32, tag="ex", name=f"ex{t}")
                exm = obp.tile([128, S], F32, tag="exm", name=f"exm{t}")
                sh = [statp.tile([128, 1], F32, tag="ss", name=f"ss{t}_{h}")
                      for h in range(2)]
                for hh in range(2):
                    nc.scalar.activation(exs[:, ts(hh, 512)],
                                         pst_halves[hh][:], Exp)
                    nc.vector.scalar_tensor_tensor(
                        exm[:, ts(hh, 512)], exs[:, ts(hh, 512)], 1.0,
                        mk[:, t, ds(hh * 512, 512)],
                        op0=Byp, op1=Mult, accum_out=sh[hh][:])
                ssum = statp.tile([128, 1], F32, tag="ss", name=f"ssa{t}")
                nc.vector.tensor_tensor(ssum[:], sh[0][:], sh[1][:],
                                        op=mybir.AluOpType.add)
                rec = statp.tile([128, 1], F32, tag="rc", name=f"rc{t}")
                nc.vector.reciprocal(rec[:], ssum[:])
                ot = obp.tile([128, S], F16, tag="ot", name=f"ot{t}")
                seng = nc.vector  # final scales; Pool regressed (SP-queue HOL)
                for hh in range(2):
                    seng.tensor_scalar_mul(
                        ot[:, ts(hh, 512)], exm[:, ts(hh, 512)], rec[:])
                    nc.sync.dma_start(
                        out=out[ts(t, 128), ds(hh * 512, 512)],
                        in_=ot[:, ts(hh, 512)])

            # ---- deferred tile t=2 on freed ppq half-slots (per-half deps)
            pb2 = [ppq.tile([128, 512], F32, tag="pq", name=f"psB2_{kh}")
                   for kh in range(2)]
            for kh in range(2):
                for jj in range(NJ):
                    nc.tensor.matmul(
                        pb2[kh][:], qp[jj][:, ts(2, 128)],
                        kp[jj][:, ts(kh, 512)],
                        start=(jj == 0), stop=(jj == NJ - 1))

            # inline-tile tails first: ready at loop end, and their psA
            # halves free up for t=3
            tail_halves(0, [psA[(0, 0)], psA[(0, 1)]])
            tail_halves(1, [psA[(1, 0)], psA[(1, 1)]])

            # ---- last tile t=3 on recycled psc half-slots, kh-major
            pb3 = [psc.tile([128, 512], F32, tag="ps", name=f"psB3_{kh}")
                   for kh in range(2)]
            for kh in range(2):
                for jj in range(NJ):
                    nc.tensor.matmul(
                        pb3[kh][:], qp[jj][:, ts(3, 128)],
                        kp[jj][:, ts(kh, 512)],
                        start=(jj == 0), stop=(jj == NJ - 1))

            tail_halves(2, pb2)
            tail_halves(3, pb3)

    nc.compile()
    return nc


_NC = None


def _get_nc():
    global _NC
    if _NC is None:
        _NC = build_nc()
    return _NC


def make_in_maps(query, key, mask, Wq, bq, Wk, bk, Wc, bc):
    query = np.asarray(query, np.float32)
    key = np.asarray(key, np.float32)
    mask = np.asarray(mask)
    Wq = np.asarray(Wq, np.float32)
    Wk = np.asarray(Wk, np.float32)
    Wc = np.asarray(Wc, np.float32)
    bq = np.asarray(bq, np.float32)
    bk = np.asarray(bk, np.float32)

    def blockdiag(W):
        blk = np.zeros((128, 128), np.float32)
        blk[0:64, 0:64] = W.T
        blk[64:128, 64:128] = W.T
        return blk

    wts = np.zeros((128, 256), np.float32)
    wts[:, 0:128] = blockdiag(Wk)
    wts[:, 128:256] = blockdiag(Wq)
    wts = wts.astype(NP_BF16)

    aux = np.zeros((128, AFREE), np.float32)
    aux[:, AOFF_BK] = np.tile(bk.reshape(-1), 2)
    aux[:, AOFF_BQ] = np.tile(bq.reshape(-1), 2)
    for j in range(NJ):
        aux[0:64, AOFF_WC + j] = Wc[0, 2 * j]
        aux[64:128, AOFF_WC + j] = Wc[0, 2 * j + 1]

    in_maps = []
    for c in range(NCORES):
        b, half = divmod(c, 2)
        s0 = half * SQ
        qh = query[b].reshape(H, S, DK)[:, s0:s0 + SQ, :]
        qTc = np.ascontiguousarray(
            qh.transpose(0, 2, 1)).reshape(NJ, 128, SQ).astype(NP_BF16)
        kh_ = key[b].reshape(H, S, DK)
        kTc = np.ascontiguousarray(
            kh_.transpose(0, 2, 1)).reshape(NJ, 128, S).astype(NP_BF16)
        mc = np.ascontiguousarray(
            mask[b, s0:s0 + SQ, :].reshape(4, 128, S)).astype(np.int8)
        in_maps.append({"qT": qTc, "kT": kTc, "msk": mc, "wts": wts,
                        "aux": aux})
    return in_maps


def kernel(query, key, mask, Wq, bq, Wk, bk, Wc, bc):
    from concourse.bass_utils import run_bass_kernel_spmd

    nc = _get_nc()
    in_maps = make_in_maps(query, key, mask, Wq, bq, Wk, bk, Wc, bc)
    res = run_bass_kernel_spmd(nc, in_maps, list(range(NCORES)))
    full = np.empty((B, S, S), np.float32)
    for c in range(NCORES):
        b, half = divmod(c, 2)
        full[b, half * SQ:(half + 1) * SQ, :] = \
            res.results[c]["out"].astype(np.float32)
    return full


# revision 5
# speedup vs baseline: 1.0279x; 1.0025x over previous
"""Trainium2 Bass kernel for nn_MHParallelAttention (B=4,S=1024,H=16,DK=64).

Sharding: 8 cores = (batch) x (query-row half); each core owns output rows
[b, s0:s0+512, :] end-to-end, no collectives.

Algebra folds:
  * sum_h Wc_h*(q_h . k_h) == (concat_h Wc_h*q_h) . (concat_h k_h): scores +
    head-combine collapse to one [512,1024]@[1024,1024]^T matmul per core,
    PSUM-accumulated over 8 chunks of 128 features.
  * bc is softmax-shift-invariant -> dropped.
  * block-diag [[W,0],[0,W]] 128x128 projection weights process a head PAIR
    per matmul.
  * softmax without max-subtraction (|logit| <= ~6 here); 0/1 int8 mask
    applied multiplicatively AFTER exp, fused with the row-sum in one DVE
    scalar_tensor_tensor pass.

Schedule (cost-model timeline 33.3us vs 39.9us for the f32r baseline):
  * all matmuls bf16: same 1 cycle/row as f32r on the PE, but half the DMA
    bytes - the DMA engines are a single serial ~17us-at-f32 resource.
  * j-loop: kt_j/qt_j DMAs in consumption order; kp proj as 2 single-bank
    [128,512] psum matmuls + per-half tanh (ppk bufs=2 so the ACT stream
    never waits); qp proj per j + tanh + DVE wc-scale (bf16 4x mode).
    Inline score accumulation for row-tiles t=0,1 into four single-bank
    half-psums, emitted with a 2-j lag so the PE never head-of-line blocks
    on the qp tanh/scale chain.
  * deferred tiles: t=2 on the ppq slots freed by the last qp tanh, t=3 on
    the psc slots freed by t=0's exp - both kh-major with per-half psum
    tiles so each half's exp chains off its own 8 matmuls.
  * tails: exp (ACT) -> masked-sum stt (DVE, accum_out) -> reciprocal ->
    per-half scale -> fp16 out DMA on the sync queue (host upcasts to f32;
    fp16 quantization ~5e-4 << 2e-2 tolerance). Tails emitted in readiness
    order so the out-DMA train streams while t=2/t=3 matmuls finish.
  * warmup: DVE memsets + 4 dummy matmuls ramp the PE p-state before real
    work; a tiny tanh absorbs the 1283ns LoadActFuncSet; wts rides the
    scalar-queue DGE in parallel with the sync queue's kt stream.

Host-side prep is layout/dtype-only; all FLOPs run on device.
"""

import os
import sys

import numpy as np

for _p in ("/opt/trn_rl_repo", "/root/.axon_site/_ro/trn_rl_repo"):
    if os.path.isdir(_p) and _p not in sys.path:
        sys.path.insert(0, _p)

import concourse.bass as bass
import concourse.mybir as mybir
import concourse.tile as tile
from concourse import bacc
from concourse.bass import ds, ts

H, DK = 16, 64
B, S = 4, 1024
SQ = 512
NCORES = 8
NJ = 8

F32 = mybir.dt.float32
F16 = mybir.dt.float16
BF16 = mybir.dt.bfloat16
I8 = mybir.dt.int8
NP_BF16 = mybir.dt.np(BF16)

# aux f32 layout along free dim: bk | bq | wc[8]
AOFF_BK, AOFF_BQ, AOFF_WC = 0, 1, 2
AFREE = 10

Tanh = mybir.ActivationFunctionType.Tanh
Exp = mybir.ActivationFunctionType.Exp
Byp = mybir.AluOpType.bypass
Mult = mybir.AluOpType.mult


def build_nc():
    nc = bacc.Bacc(None, target_bir_lowering=False, debug=False)

    qT = nc.dram_tensor("qT", [NJ, 128, SQ], BF16, kind="ExternalInput")
    kT = nc.dram_tensor("kT", [NJ, 128, S], BF16, kind="ExternalInput")
    msk = nc.dram_tensor("msk", [4, 128, S], I8, kind="ExternalInput")
    wts = nc.dram_tensor("wts", [128, 384], BF16, kind="ExternalInput")
    mska = nc.dram_tensor("mska", [128, 512], BF16, kind="ExternalInput")
    aux = nc.dram_tensor("aux", [128, AFREE], F32, kind="ExternalInput")
    out = nc.dram_tensor("out", [SQ, S], F16, kind="ExternalOutput")

    with tile.TileContext(nc) as tc:
        with (
            tc.tile_pool(name="const", bufs=1) as cst,
            tc.tile_pool(name="kin", bufs=3) as kin,
            tc.tile_pool(name="qin", bufs=3) as qin,
            tc.tile_pool(name="kpp", bufs=1) as kpp,
            tc.tile_pool(name="qpp", bufs=1) as qpp,
            tc.tile_pool(name="tqp", bufs=3) as tqp,
            tc.tile_pool(name="mrow", bufs=1) as mrp,
            tc.tile_pool(name="soft", bufs=4) as softp,
            tc.tile_pool(name="stat", bufs=8) as statp,
            tc.tile_pool(name="obuf", bufs=4) as obp,
            tc.tile_pool(name="ppk", bufs=2, space="PSUM") as ppk,
            tc.tile_pool(name="ppq", bufs=2, space="PSUM") as ppq,
            tc.tile_pool(name="psc", bufs=4, space="PSUM") as psc,
        ):
            # ---- warmup junk first: DVE memsets at t~0, then dummy
            # matmuls (p-state ramp) and a tiny tanh (absorbs the 1283ns
            # LoadActFuncSet before the real pipeline).
            jst = cst.tile([128, 128], BF16, tag="jst", name="jst")
            nc.vector.memset(jst[:], 0.25)
            jmv = cst.tile([128, 512], BF16, tag="jmv", name="jmv")
            nc.vector.memset(jmv[:], 0.25)
            jact = cst.tile([128, 16], F32, tag="jact", name="jact")
            nc.scalar.activation(jact[:], jst[:, 0:16], Tanh)

            # ---- small constants at the FRONT of the sync queue (land
            # before kt0; first kp matmul needs wkb)
            wts_sb = cst.tile([128, 384], BF16, tag="wts", name="wts_sb")
            nc.scalar.dma_start(out=wts_sb[:], in_=wts[:])
            aux_sb = cst.tile([128, AFREE], F32, tag="aux", name="aux_sb")
            nc.sync.dma_start(out=aux_sb[:], in_=aux[:])
            wkb = wts_sb[:, 0:128]
            wqb = wts_sb[:, 128:256]
            identb = wts_sb[:, 256:384]
            bkb = aux_sb[:, AOFF_BK:AOFF_BK + 1]
            bqb = aux_sb[:, AOFF_BQ:AOFF_BQ + 1]
            wcb = aux_sb[:, AOFF_WC:AOFF_WC + NJ]

            mk = mrp.tile([128, 4, S], I8, tag="mk", name="mk")

            kp = [kpp.tile([128, S], BF16, tag=f"kp{j}", name=f"kp{j}")
                  for j in range(NJ)]
            qp = [qpp.tile([128, SQ], BF16, tag=f"qp{j}", name=f"qp{j}")
                  for j in range(NJ)]

            warm = ppk.tile([128, 512], F32, tag="pk", name="warm")
            for w in range(4):
                nc.tensor.matmul(warm[:], jst[:], jmv[:],
                                 start=True, stop=True)

            # inline score tiles t=0,1 as per-half psum tiles (finer deps)
            psA = {(t, kh): psc.tile([128, 512], F32, tag="ps",
                                     name=f"psA_{t}_{kh}")
                   for t in range(2) for kh in range(2)}

            # ---- j-loop: input DMAs in consumption order on the sync
            # queue; unfused per-half kp proj + per-j qp proj (all [128,512]
            # single-bank psums, bufs=2 rotation); inline score matmuls for
            # t=0,1 emitted with a 3-j lag so the PE never head-of-line
            # blocks on the qp tanh/scale chain.
            def emit_scores(jj):
                for t in range(2):
                    for kh in range(2):
                        nc.tensor.matmul(
                            psA[(t, kh)][:], qp[jj][:, ts(t, 128)],
                            kp[jj][:, ts(kh, 512)],
                            start=(jj == 0), stop=(jj == NJ - 1))

            for j in range(NJ):
                kt = kin.tile([128, S], BF16, tag="kt", name=f"kt{j}")
                if j == 0:
                    nc.sync.dma_start(out=kt[:, 0:512], in_=kT[0][:, 0:512])
                    nc.sync.dma_start(out=kt[:, 512:1024],
                                      in_=kT[0][:, 512:1024])
                else:
                    nc.sync.dma_start(out=kt[:], in_=kT[j])
                qt = qin.tile([128, SQ], BF16, tag="qt", name=f"qt{j}")
                nc.sync.dma_start(out=qt[:], in_=qT[j])
                for kh in range(2):
                    pk = ppk.tile([128, 512], F32, tag="pk",
                                  name=f"pk{j}_{kh}")
                    nc.tensor.matmul(pk[:], wkb, kt[:, ts(kh, 512)],
                                     start=True, stop=True)
                    nc.scalar.activation(kp[j][:, ts(kh, 512)], pk[:], Tanh,
                                         bias=bkb)

                pqj = ppq.tile([128, 512], F32, tag="pq", name=f"pq{j}")
                nc.tensor.matmul(pqj[:], wqb, qt[:], start=True, stop=True)
                tq = tqp.tile([128, SQ], BF16, tag="tq", name=f"tq{j}")
                nc.scalar.activation(tq[:], pqj[:], Tanh, bias=bqb)
                nc.vector.tensor_scalar_mul(qp[j][:], tq[:], wcb[:, j:j + 1])

                if j >= 3:
                    emit_scores(j - 3)
            for jj in (NJ - 3, NJ - 2, NJ - 1):
                emit_scores(jj)

            # mask arrives after the input stream; needed from the first tail
            nc.sync.dma_start(out=mk[:], in_=msk[:].rearrange("t p k -> p t k"))
            mka_sb = mrp.tile([128, 512], BF16, tag="mka", name="mka_sb")
            nc.sync.dma_start(out=mka_sb[:], in_=mska[:])

            def tail_halves(t, pst_halves):
                exs = softp.tile([128, S], F32, tag="ex", name=f"ex{t}")
                exm = obp.tile([128, S], F32, tag="exm", name=f"exm{t}")
                sh = [statp.tile([128, 1], F32, tag="ss", name=f"ss{t}_{h}")
                      for h in range(2)]
                for hh in range(2):
                    nc.scalar.activation(exs[:, ts(hh, 512)],
                                         pst_halves[hh][:], Exp)
                    nc.vector.scalar_tensor_tensor(
                        exm[:, ts(hh, 512)], exs[:, ts(hh, 512)], 1.0,
                        mk[:, t, ds(hh * 512, 512)],
                        op0=Byp, op1=Mult, accum_out=sh[hh][:])
                ssum = statp.tile([128, 1], F32, tag="ss", name=f"ssa{t}")
                nc.vector.tensor_tensor(ssum[:], sh[0][:], sh[1][:],
                                        op=mybir.AluOpType.add)
                rec = statp.tile([128, 1], F32, tag="rc", name=f"rc{t}")
                nc.vector.reciprocal(rec[:], ssum[:])
                ot = obp.tile([128, S], F16, tag="ot", name=f"ot{t}")
                for hh in range(2):
                    nc.vector.tensor_scalar_mul(
                        ot[:, ts(hh, 512)], exm[:, ts(hh, 512)], rec[:])
                nc.sync.dma_start(out=out[ts(t, 128), :], in_=ot[:])

            # ---- deferred tile t=2 on freed ppq half-slots (per-half deps)
            pb2 = [ppq.tile([128, 512], F32, tag="pq", name=f"psB2_{kh}")
                   for kh in range(2)]
            for kh in range(2):
                for jj in range(NJ):
                    nc.tensor.matmul(
                        pb2[kh][:], qp[jj][:, ts(2, 128)],
                        kp[jj][:, ts(kh, 512)],
                        start=(jj == 0), stop=(jj == NJ - 1))

            # inline-tile tails first: ready at loop end, and their psA
            # halves free up for t=3
            tail_halves(0, [psA[(0, 0)], psA[(0, 1)]])
            tail_halves(1, [psA[(1, 0)], psA[(1, 1)]])

            # ---- last tile t=3 on recycled psc half-slots, kh-major
            pb3 = [psc.tile([128, 512], F32, tag="ps", name=f"psB3_{kh}")
                   for kh in range(2)]
            for kh in range(2):
                for jj in range(NJ):
                    nc.tensor.matmul(
                        pb3[kh][:], qp[jj][:, ts(3, 128)],
                        kp[jj][:, ts(kh, 512)],
                        start=(jj == 0),
                        stop=(jj == NJ - 1 and kh == 0))
            # additive {0,-30000} mask folded into the kh1 psum through the
            # PE (identity stationary), so exp+accum_out yields the masked
            # row-sum directly - no DVE pass on the terminal chain
            nc.tensor.matmul(pb3[1][:], identb, mka_sb[:],
                             start=False, stop=True)

            tail_halves(2, pb2)

            # t=3 tail: kh0 via the stt path, kh1 via exp+accum on the
            # pre-masked psum
            exs3 = softp.tile([128, S], F32, tag="ex", name="ex3")
            exm3 = obp.tile([128, S], F32, tag="exm", name="exm3")
            s3 = [statp.tile([128, 1], F32, tag="ss", name=f"ss3_{h}")
                  for h in range(2)]
            nc.scalar.activation(exs3[:, ts(0, 512)], pb3[0][:], Exp)
            nc.vector.scalar_tensor_tensor(
                exm3[:, ts(0, 512)], exs3[:, ts(0, 512)], 1.0,
                mk[:, 3, ds(0, 512)],
                op0=Byp, op1=Mult, accum_out=s3[0][:])
            nc.scalar.activation(exs3[:, ts(1, 512)], pb3[1][:], Exp,
                                 accum_out=s3[1][:])
            ssum3 = statp.tile([128, 1], F32, tag="ss", name="ssa3")
            nc.vector.tensor_tensor(ssum3[:], s3[0][:], s3[1][:],
                                    op=mybir.AluOpType.add)
            rec3 = statp.tile([128, 1], F32, tag="rc", name="rc3")
            nc.vector.reciprocal(rec3[:], ssum3[:])
            ot3 = obp.tile([128, S], F16, tag="ot", name="ot3")
            nc.vector.tensor_scalar_mul(
                ot3[:, ts(0, 512)], exm3[:, ts(0, 512)], rec3[:])
            nc.vector.tensor_scalar_mul(
                ot3[:, ts(1, 512)], exs3[:, ts(1, 512)], rec3[:])
            nc.sync.dma_start(out=out[ts(3, 128), :], in_=ot3[:])

    nc.compile()
    return nc


_NC = None


def _get_nc():
    global _NC
    if _NC is None:
        _NC = build_nc()
    return _NC


def make_in_maps(query, key, mask, Wq, bq, Wk, bk, Wc, bc):
    query = np.asarray(query, np.float32)
    key = np.asarray(key, np.float32)
    mask = np.asarray(mask)
    Wq = np.asarray(Wq, np.float32)
    Wk = np.asarray(Wk, np.float32)
    Wc = np.asarray(Wc, np.float32)
    bq = np.asarray(bq, np.float32)
    bk = np.asarray(bk, np.float32)

    def blockdiag(W):
        blk = np.zeros((128, 128), np.float32)
        blk[0:64, 0:64] = W.T
        blk[64:128, 64:128] = W.T
        return blk

    wts = np.zeros((128, 384), np.float32)
    wts[:, 0:128] = blockdiag(Wk)
    wts[:, 128:256] = blockdiag(Wq)
    wts[:, 256:384] = np.eye(128, dtype=np.float32)
    wts = wts.astype(NP_BF16)

    aux = np.zeros((128, AFREE), np.float32)
    aux[:, AOFF_BK] = np.tile(bk.reshape(-1), 2)
    aux[:, AOFF_BQ] = np.tile(bq.reshape(-1), 2)
    for j in range(NJ):
        aux[0:64, AOFF_WC + j] = Wc[0, 2 * j]
        aux[64:128, AOFF_WC + j] = Wc[0, 2 * j + 1]

    in_maps = []
    for c in range(NCORES):
        b, half = divmod(c, 2)
        s0 = half * SQ
        qh = query[b].reshape(H, S, DK)[:, s0:s0 + SQ, :]
        qTc = np.ascontiguousarray(
            qh.transpose(0, 2, 1)).reshape(NJ, 128, SQ).astype(NP_BF16)
        kh_ = key[b].reshape(H, S, DK)
        kTc = np.ascontiguousarray(
            kh_.transpose(0, 2, 1)).reshape(NJ, 128, S).astype(NP_BF16)

        mc = np.ascontiguousarray(
            mask[b, s0:s0 + SQ, :].reshape(4, 128, S)).astype(np.int8)
        mka = ((mask[b, s0 + 384:s0 + 512, 512:1024] == 0)
               .astype(np.float32) * -30000.0).astype(NP_BF16)
        in_maps.append({"qT": qTc, "kT": kTc, "msk": mc, "wts": wts,
                        "aux": aux, "mska": mka})
    return in_maps


def kernel(query, key, mask, Wq, bq, Wk, bk, Wc, bc):
    from concourse.bass_utils import run_bass_kernel_spmd

    nc = _get_nc()
    in_maps = make_in_maps(query, key, mask, Wq, bq, Wk, bk, Wc, bc)
    res = run_bass_kernel_spmd(nc, in_maps, list(range(NCORES)))
    full = np.empty((B, S, S), np.float32)
    for c in range(NCORES):
        b, half = divmod(c, 2)
        full[b, half * SQ:(half + 1) * SQ, :] = \
            res.results[c]["out"].astype(np.float32)
    return full


# revision 7
# speedup vs baseline: 1.0437x; 1.0153x over previous
"""Trainium2 Bass kernel for nn_MHParallelAttention (B=4,S=1024,H=16,DK=64).

Sharding: 8 cores = (batch) x (query-row half); each core owns output rows
[b, s0:s0+512, :] end-to-end, no collectives.

Algebra folds:
  * sum_h Wc_h*(q_h . k_h) == (concat_h Wc_h*q_h) . (concat_h k_h): scores +
    head-combine collapse to one [512,1024]@[1024,1024]^T matmul per core,
    PSUM-accumulated over 8 chunks of 128 features.
  * bc is softmax-shift-invariant -> dropped.
  * block-diag [[W,0],[0,W]] 128x128 projection weights process a head PAIR
    per matmul.
  * softmax without max-subtraction (|logit| <= ~6 here); 0/1 int8 mask
    applied multiplicatively AFTER exp, fused with the row-sum in one DVE
    scalar_tensor_tensor pass.

Schedule (cost-model timeline 32.4us vs 39.9us for the f32r baseline):
  * all matmuls bf16: same 1 cycle/row as f32r on the PE, but half the DMA
    bytes - the DMA engines are a single serial ~17us-at-f32 resource.
  * j-loop: kt_j/qt_j DMAs in consumption order; kp proj as 2 single-bank
    [128,512] psum matmuls + per-half tanh (ppk bufs=2 so the ACT stream
    never waits); qp proj per j + tanh + DVE wc-scale (bf16 4x mode).
    Inline score accumulation for row-tiles t=0,1 into four single-bank
    half-psums, emitted with a 2-j lag so the PE never head-of-line blocks
    on the qp tanh/scale chain.
  * t=2 kh0 accumulates inline on the bank freed by making the qp-proj
    psum single-buffered (that chain has 1.8us of ACT slack per step). Deferred after the loop: t2-kh1 on the ppq
    slot, t3 halves on psc slots freed by t=0's exps - per-half psum tiles
    so each half's exp chains off its own 8 matmuls.
  * tails: exp (ACT) -> masked-sum stt (DVE, accum_out) -> reciprocal ->
    per-half scale -> ONE fp16 [128,1024] out DMA per tile on the sync
    queue (HWDGE descriptor-gen is a serial 625ns/DMA device - fewer,
    bigger DMAs; host upcasts fp16->f32, quantization ~5e-4 << 2e-2 tol).
  * terminal chain (t=3 kh1): the 0/-30000 additive mask is folded into
    the psum by an identity-stationary matmul (start=True initializer), so
    exp+accum_out yields the masked row-sum directly - no DVE masked-sum
    pass on the critical chain.
  * warmup: DVE memsets + 4 dummy matmuls ramp the PE p-state before real
    work; a tiny tanh absorbs the 1283ns LoadActFuncSet; wts rides the
    scalar-queue DGE in parallel with the sync queue's kt stream.

Host-side prep is layout/dtype-only; all FLOPs run on device.
"""

import os
import sys

import numpy as np

for _p in ("/opt/trn_rl_repo", "/root/.axon_site/_ro/trn_rl_repo"):
    if os.path.isdir(_p) and _p not in sys.path:
        sys.path.insert(0, _p)

import concourse.bass as bass
import concourse.mybir as mybir
import concourse.tile as tile
from concourse import bacc
from concourse.bass import ds, ts

H, DK = 16, 64
B, S = 4, 1024
SQ = 512
NCORES = 8
NJ = 8

F32 = mybir.dt.float32
F16 = mybir.dt.float16
BF16 = mybir.dt.bfloat16
I8 = mybir.dt.int8
NP_BF16 = mybir.dt.np(BF16)

# aux f32 layout along free dim: bk | bq | wc[8]
AOFF_BK, AOFF_BQ, AOFF_WC = 0, 1, 2
AFREE = 10

Tanh = mybir.ActivationFunctionType.Tanh
Exp = mybir.ActivationFunctionType.Exp
Byp = mybir.AluOpType.bypass
Mult = mybir.AluOpType.mult


def build_nc():
    nc = bacc.Bacc(None, target_bir_lowering=False, debug=False)

    qT = nc.dram_tensor("qT", [NJ, 128, SQ], BF16, kind="ExternalInput")
    kT = nc.dram_tensor("kT", [NJ, 128, S], BF16, kind="ExternalInput")
    msk = nc.dram_tensor("msk", [4, 128, S], I8, kind="ExternalInput")
    wts = nc.dram_tensor("wts", [128, 384], BF16, kind="ExternalInput")
    mska = nc.dram_tensor("mska", [128, 512], BF16, kind="ExternalInput")
    aux = nc.dram_tensor("aux", [128, AFREE], F32, kind="ExternalInput")
    out = nc.dram_tensor("out", [SQ, S], F16, kind="ExternalOutput")

    with tile.TileContext(nc) as tc:
        with (
            tc.tile_pool(name="const", bufs=1) as cst,
            tc.tile_pool(name="kin", bufs=3) as kin,
            tc.tile_pool(name="qin", bufs=3) as qin,
            tc.tile_pool(name="kpp", bufs=1) as kpp,
            tc.tile_pool(name="qpp", bufs=1) as qpp,
            tc.tile_pool(name="tqp", bufs=3) as tqp,
            tc.tile_pool(name="mrow", bufs=1) as mrp,
            tc.tile_pool(name="soft", bufs=4) as softp,
            tc.tile_pool(name="stat", bufs=8) as statp,
            tc.tile_pool(name="obuf", bufs=4) as obp,
            tc.tile_pool(name="ppk", bufs=2, space="PSUM") as ppk,
            tc.tile_pool(name="ppq", bufs=1, space="PSUM") as ppq,
            tc.tile_pool(name="pb2p", bufs=1, space="PSUM") as pb2p,
            tc.tile_pool(name="psc", bufs=4, space="PSUM") as psc,
        ):
            # ---- warmup junk first: DVE memsets at t~0, then dummy
            # matmuls (p-state ramp) and a tiny tanh (absorbs the 1283ns
            # LoadActFuncSet before the real pipeline).
            jst = cst.tile([128, 128], BF16, tag="jst", name="jst")
            nc.vector.memset(jst[:], 0.25)
            jmv = cst.tile([128, 512], BF16, tag="jmv", name="jmv")
            nc.vector.memset(jmv[:], 0.25)
            jact = cst.tile([128, 16], F32, tag="jact", name="jact")
            nc.scalar.activation(jact[:], jst[:, 0:16], Tanh)

            # ---- small constants at the FRONT of the sync queue (land
            # before kt0; first kp matmul needs wkb)
            wts_sb = cst.tile([128, 384], BF16, tag="wts", name="wts_sb")
            nc.scalar.dma_start(out=wts_sb[:], in_=wts[:])
            aux_sb = cst.tile([128, AFREE], F32, tag="aux", name="aux_sb")
            nc.sync.dma_start(out=aux_sb[:], in_=aux[:])
            wkb = wts_sb[:, 0:128]
            wqb = wts_sb[:, 128:256]
            identb = wts_sb[:, 256:384]
            bkb = aux_sb[:, AOFF_BK:AOFF_BK + 1]
            bqb = aux_sb[:, AOFF_BQ:AOFF_BQ + 1]
            wcb = aux_sb[:, AOFF_WC:AOFF_WC + NJ]

            mk = mrp.tile([128, 4, S], I8, tag="mk", name="mk")

            kp = [kpp.tile([128, S], BF16, tag=f"kp{j}", name=f"kp{j}")
                  for j in range(NJ)]
            qp = [qpp.tile([128, SQ], BF16, tag=f"qp{j}", name=f"qp{j}")
                  for j in range(NJ)]

            warm = ppk.tile([128, 512], F32, tag="pk", name="warm")
            for w in range(4):
                nc.tensor.matmul(warm[:], jst[:], jmv[:],
                                 start=True, stop=True)

            # inline score tiles t=0,1 as per-half psum tiles (finer deps)
            psA = {(t, kh): psc.tile([128, 512], F32, tag="ps",
                                     name=f"psA_{t}_{kh}")
                   for t in range(2) for kh in range(2)}

            # ---- j-loop: input DMAs in consumption order on the sync
            # queue; unfused per-half kp proj + per-j qp proj (all [128,512]
            # single-bank psums, bufs=2 rotation); inline score matmuls for
            # t=0,1 emitted with a 3-j lag so the PE never head-of-line
            # blocks on the qp tanh/scale chain.
            pb2h0 = pb2p.tile([128, 512], F32, tag="p2", name="pb2h0")

            def emit_scores(jj):
                for t in range(2):
                    for kh in range(2):
                        nc.tensor.matmul(
                            psA[(t, kh)][:], qp[jj][:, ts(t, 128)],
                            kp[jj][:, ts(kh, 512)],
                            start=(jj == 0), stop=(jj == NJ - 1))
                nc.tensor.matmul(
                    pb2h0[:], qp[jj][:, ts(2, 128)], kp[jj][:, ts(0, 512)],
                    start=(jj == 0), stop=(jj == NJ - 1))

            for j in range(NJ):
                kt = kin.tile([128, S], BF16, tag="kt", name=f"kt{j}")
                if j == 0:
                    nc.sync.dma_start(out=kt[:, 0:512], in_=kT[0][:, 0:512])
                    nc.sync.dma_start(out=kt[:, 512:1024],
                                      in_=kT[0][:, 512:1024])
                else:
                    nc.sync.dma_start(out=kt[:], in_=kT[j])
                qt = qin.tile([128, SQ], BF16, tag="qt", name=f"qt{j}")
                nc.sync.dma_start(out=qt[:], in_=qT[j])
                for kh in range(2):
                    pk = ppk.tile([128, 512], F32, tag="pk",
                                  name=f"pk{j}_{kh}")
                    nc.tensor.matmul(pk[:], wkb, kt[:, ts(kh, 512)],
                                     start=True, stop=True)
                    nc.scalar.activation(kp[j][:, ts(kh, 512)], pk[:], Tanh,
                                         bias=bkb)

                pqj = ppq.tile([128, 512], F32, tag="pq", name=f"pq{j}")
                nc.tensor.matmul(pqj[:], wqb, qt[:], start=True, stop=True)
                tq = tqp.tile([128, SQ], BF16, tag="tq", name=f"tq{j}")
                nc.scalar.activation(tq[:], pqj[:], Tanh, bias=bqb)
                nc.vector.tensor_scalar_mul(qp[j][:], tq[:], wcb[:, j:j + 1])

                if j >= 3:
                    emit_scores(j - 3)
            for jj in (NJ - 3, NJ - 2, NJ - 1):
                emit_scores(jj)

            # mask arrives after the input stream; needed from the first tail
            nc.sync.dma_start(out=mk[:], in_=msk[:].rearrange("t p k -> p t k"))
            mka_sb = mrp.tile([128, 512], BF16, tag="mka", name="mka_sb")
            nc.sync.dma_start(out=mka_sb[:], in_=mska[:])

            def tail_halves(t, pst_halves):
                exs = softp.tile([128, S], F32, tag="ex", name=f"ex{t}")
                exm = obp.tile([128, S], F32, tag="exm", name=f"exm{t}")
                sh = [statp.tile([128, 1], F32, tag="ss", name=f"ss{t}_{h}")
                      for h in range(2)]
                for hh in range(2):
                    nc.scalar.activation(exs[:, ts(hh, 512)],
                                         pst_halves[hh][:], Exp)
                    nc.vector.scalar_tensor_tensor(
                        exm[:, ts(hh, 512)], exs[:, ts(hh, 512)], 1.0,
                        mk[:, t, ds(hh * 512, 512)],
                        op0=Byp, op1=Mult, accum_out=sh[hh][:])
                ssum = statp.tile([128, 1], F32, tag="ss", name=f"ssa{t}")
                nc.vector.tensor_tensor(ssum[:], sh[0][:], sh[1][:],
                                        op=mybir.AluOpType.add)
                rec = statp.tile([128, 1], F32, tag="rc", name=f"rc{t}")
                nc.vector.reciprocal(rec[:], ssum[:])
                ot = obp.tile([128, S], F16, tag="ot", name=f"ot{t}")
                for hh in range(2):
                    nc.vector.tensor_scalar_mul(
                        ot[:, ts(hh, 512)], exm[:, ts(hh, 512)], rec[:])
                nc.sync.dma_start(out=out[ts(t, 128), :], in_=ot[:])

            # ---- deferred t=2 kh1 on the ppq slot freed by the last tanh
            pb2h1 = ppq.tile([128, 512], F32, tag="pq", name="psB2_1")
            for jj in range(NJ):
                nc.tensor.matmul(
                    pb2h1[:], qp[jj][:, ts(2, 128)], kp[jj][:, ts(1, 512)],
                    start=(jj == 0), stop=(jj == NJ - 1))
            pb2 = [pb2h0, pb2h1]

            # inline-tile tails first: ready at loop end, and their psA
            # halves free up for t=3
            tail_halves(0, [psA[(0, 0)], psA[(0, 1)]])
            tail_halves(1, [psA[(1, 0)], psA[(1, 1)]])

            # ---- last tile t=3 on recycled psc half-slots, kh-major
            pb3 = [psc.tile([128, 512], F32, tag="ps", name=f"psB3_{kh}")
                   for kh in range(2)]
            for jj in range(NJ):
                nc.tensor.matmul(
                    pb3[0][:], qp[jj][:, ts(3, 128)], kp[jj][:, ts(0, 512)],
                    start=(jj == 0), stop=(jj == NJ - 1))
            # additive {0,-30000} mask initializes the kh1 psum through the
            # PE (identity stationary, start=True), so exp+accum_out yields
            # the masked row-sum directly - no DVE pass and no extra matmul
            # on the terminal chain
            nc.tensor.matmul(pb3[1][:], identb, mka_sb[:],
                             start=True, stop=False)
            for jj in range(NJ):
                nc.tensor.matmul(
                    pb3[1][:], qp[jj][:, ts(3, 128)], kp[jj][:, ts(1, 512)],
                    start=False, stop=(jj == NJ - 1))

            tail_halves(2, pb2)

            # t=3 tail: kh0 via the stt path, kh1 via exp+accum on the
            # pre-masked psum
            exs3 = softp.tile([128, S], F32, tag="ex", name="ex3")
            exm3 = obp.tile([128, S], F32, tag="exm", name="exm3")
            s3 = [statp.tile([128, 1], F32, tag="ss", name=f"ss3_{h}")
                  for h in range(2)]
            nc.scalar.activation(exs3[:, ts(0, 512)], pb3[0][:], Exp)
            nc.vector.scalar_tensor_tensor(
                exm3[:, ts(0, 512)], exs3[:, ts(0, 512)], 1.0,
                mk[:, 3, ds(0, 512)],
                op0=Byp, op1=Mult, accum_out=s3[0][:])
            nc.scalar.activation(exs3[:, ts(1, 512)], pb3[1][:], Exp,
                                 accum_out=s3[1][:])
            ssum3 = statp.tile([128, 1], F32, tag="ss", name="ssa3")
            nc.vector.tensor_tensor(ssum3[:], s3[0][:], s3[1][:],
                                    op=mybir.AluOpType.add)
            rec3 = statp.tile([128, 1], F32, tag="rc", name="rc3")
            nc.vector.reciprocal(rec3[:], ssum3[:])
            ot3 = obp.tile([128, S], F16, tag="ot", name="ot3")
            nc.vector.tensor_scalar_mul(
                ot3[:, ts(0, 512)], exm3[:, ts(0, 512)], rec3[:])
            nc.vector.tensor_scalar_mul(
                ot3[:, ts(1, 512)], exs3[:, ts(1, 512)], rec3[:])
            nc.sync.dma_start(out=out[ts(3, 128), :], in_=ot3[:])

    nc.compile()
    return nc


_NC = None


def _get_nc():
    global _NC
    if _NC is None:
        _NC = build_nc()
    return _NC


def make_in_maps(query, key, mask, Wq, bq, Wk, bk, Wc, bc):
    query = np.asarray(query, np.float32)
    key = np.asarray(key, np.float32)
    mask = np.asarray(mask)
    Wq = np.asarray(Wq, np.float32)
    Wk = np.asarray(Wk, np.float32)
    Wc = np.asarray(Wc, np.float32)
    bq = np.asarray(bq, np.float32)
    bk = np.asarray(bk, np.float32)

    def blockdiag(W):
        blk = np.zeros((128, 128), np.float32)
        blk[0:64, 0:64] = W.T
        blk[64:128, 64:128] = W.T
        return blk

    wts = np.zeros((128, 384), np.float32)
    wts[:, 0:128] = blockdiag(Wk)
    wts[:, 128:256] = blockdiag(Wq)
    wts[:, 256:384] = np.eye(128, dtype=np.float32)
    wts = wts.astype(NP_BF16)

    aux = np.zeros((128, AFREE), np.float32)
    aux[:, AOFF_BK] = np.tile(bk.reshape(-1), 2)
    aux[:, AOFF_BQ] = np.tile(bq.reshape(-1), 2)
    for j in range(NJ):
        aux[0:64, AOFF_WC + j] = Wc[0, 2 * j]
        aux[64:128, AOFF_WC + j] = Wc[0, 2 * j + 1]

    in_maps = []
    for c in range(NCORES):
        b, half = divmod(c, 2)
        s0 = half * SQ
        qh = query[b].reshape(H, S, DK)[:, s0:s0 + SQ, :]
        qTc = np.ascontiguousarray(
            qh.transpose(0, 2, 1)).reshape(NJ, 128, SQ).astype(NP_BF16)
        kh_ = key[b].reshape(H, S, DK)
        kTc = np.ascontiguousarray(
            kh_.transpose(0, 2, 1)).reshape(NJ, 128, S).astype(NP_BF16)

        mc = np.ascontiguousarray(
            mask[b, s0:s0 + SQ, :].reshape(4, 128, S)).astype(np.int8)
        mka = ((mask[b, s0 + 384:s0 + 512, 512:1024] == 0)
               .astype(np.float32) * -30000.0).astype(NP_BF16)
        in_maps.append({"qT": qTc, "kT": kTc, "msk": mc, "wts": wts,
                        "aux": aux, "mska": mka})
    return in_maps


def kernel(query, key, mask, Wq, bq, Wk, bk, Wc, bc):
    from concourse.bass_utils import run_bass_kernel_spmd

    nc = _get_nc()
    in_maps = make_in_maps(query, key, mask, Wq, bq, Wk, bk, Wc, bc)
    res = run_bass_kernel_spmd(nc, in_maps, list(range(NCORES)))
    full = np.empty((B, S, S), np.float32)
    for c in range(NCORES):
        b, half = divmod(c, 2)
        full[b, half * SQ:(half + 1) * SQ, :] = \
            res.results[c]["out"].astype(np.float32)
    return full


# revision 8
# speedup vs baseline: 1.0597x; 1.0153x over previous
"""Trainium2 Bass kernel for nn_MHParallelAttention (B=4,S=1024,H=16,DK=64).

Sharding: 8 cores = (batch) x (query-row half); each core owns output rows
[b, s0:s0+512, :] end-to-end, no collectives.

Algebra folds:
  * sum_h Wc_h*(q_h . k_h) == (concat_h Wc_h*q_h) . (concat_h k_h): scores +
    head-combine collapse to one [512,1024]@[1024,1024]^T matmul per core,
    PSUM-accumulated over 8 chunks of 128 features.
  * bc is softmax-shift-invariant -> dropped.
  * block-diag [[W,0],[0,W]] 128x128 projection weights process a head PAIR
    per matmul.
  * softmax without max-subtraction (|logit| <= ~6 here); 0/1 int8 mask
    applied multiplicatively AFTER exp, fused with the row-sum in one DVE
    scalar_tensor_tensor pass.

Schedule (cost-model timeline 31.9us vs 39.9us for the f32r baseline):
  * all matmuls bf16: same 1 cycle/row as f32r on the PE, but half the DMA
    bytes - the DMA engines are a single serial ~17us-at-f32 resource.
  * j-loop: kt_j/qt_j DMAs in consumption order; kp proj as 2 matmuls
    into one [128,1024] 2-bank psum + ONE fused tanh (the per-j qp tanh
    fills the single-buffered rotation window, keeping ACT gapless); qp
    proj per j + tanh + DVE wc-scale (bf16 4x mode).
    Inline score accumulation for row-tiles t=0,1 into four single-bank
    half-psums, emitted with a 2-j lag so the PE never head-of-line blocks
    on the qp tanh/scale chain.
  * t=2 kh0 accumulates inline on the bank freed by making the qp-proj
    psum single-buffered (that chain has 1.8us of ACT slack per step). Deferred after the loop: t2-kh1 on the ppq
    slot, t3 halves on psc slots freed by t=0's exps - per-half psum tiles
    so each half's exp chains off its own 8 matmuls.
  * tails: exp (ACT) -> masked-sum stt (DVE, accum_out) -> reciprocal ->
    per-half scale -> ONE fp16 [128,1024] out DMA per tile on the sync
    queue (HWDGE descriptor-gen is a serial 625ns/DMA device - fewer,
    bigger DMAs; host upcasts fp16->f32, quantization ~5e-4 << 2e-2 tol).
  * terminal chain (t=3 kh1): the 0/-30000 additive mask is folded into
    the psum by an identity-stationary matmul (start=True initializer), so
    exp+accum_out yields the masked row-sum directly - no DVE masked-sum
    pass on the critical chain.
  * warmup: DVE memsets + 4 dummy matmuls ramp the PE p-state before real
    work; a tiny tanh absorbs the 1283ns LoadActFuncSet; wts rides the
    scalar-queue DGE in parallel with the sync queue's kt stream.

Host-side prep is layout/dtype-only; all FLOPs run on device.
"""

import os
import sys

import numpy as np

for _p in ("/opt/trn_rl_repo", "/root/.axon_site/_ro/trn_rl_repo"):
    if os.path.isdir(_p) and _p not in sys.path:
        sys.path.insert(0, _p)

import concourse.bass as bass
import concourse.mybir as mybir
import concourse.tile as tile
from concourse import bacc
from concourse.bass import ds, ts

H, DK = 16, 64
B, S = 4, 1024
SQ = 512
NCORES = 8
NJ = 8

F32 = mybir.dt.float32
F16 = mybir.dt.float16
BF16 = mybir.dt.bfloat16
I8 = mybir.dt.int8
NP_BF16 = mybir.dt.np(BF16)

# aux f32 layout along free dim: bk | bq | wc[8]
AOFF_BK, AOFF_BQ, AOFF_WC = 0, 1, 2
AFREE = 10

Tanh = mybir.ActivationFunctionType.Tanh
Exp = mybir.ActivationFunctionType.Exp
Byp = mybir.AluOpType.bypass
Mult = mybir.AluOpType.mult


def build_nc():
    nc = bacc.Bacc(None, target_bir_lowering=False, debug=False)

    qT = nc.dram_tensor("qT", [NJ, 128, SQ], BF16, kind="ExternalInput")
    kT = nc.dram_tensor("kT", [NJ, 128, S], BF16, kind="ExternalInput")
    msk = nc.dram_tensor("msk", [4, 128, S], I8, kind="ExternalInput")
    wts = nc.dram_tensor("wts", [128, 384], BF16, kind="ExternalInput")
    mska = nc.dram_tensor("mska", [128, 512], BF16, kind="ExternalInput")
    aux = nc.dram_tensor("aux", [128, AFREE], F32, kind="ExternalInput")
    out = nc.dram_tensor("out", [SQ, S], F16, kind="ExternalOutput")

    with tile.TileContext(nc) as tc:
        with (
            tc.tile_pool(name="const", bufs=1) as cst,
            tc.tile_pool(name="kin", bufs=3) as kin,
            tc.tile_pool(name="qin", bufs=3) as qin,
            tc.tile_pool(name="kpp", bufs=1) as kpp,
            tc.tile_pool(name="qpp", bufs=1) as qpp,
            tc.tile_pool(name="tqp", bufs=3) as tqp,
            tc.tile_pool(name="mrow", bufs=1) as mrp,
            tc.tile_pool(name="soft", bufs=4) as softp,
            tc.tile_pool(name="stat", bufs=8) as statp,
            tc.tile_pool(name="obuf", bufs=4) as obp,
            tc.tile_pool(name="ppk", bufs=1, space="PSUM") as ppk,
            tc.tile_pool(name="ppq", bufs=1, space="PSUM") as ppq,
            tc.tile_pool(name="pb2p", bufs=1, space="PSUM") as pb2p,
            tc.tile_pool(name="psc", bufs=4, space="PSUM") as psc,
        ):
            # ---- warmup junk first: DVE memsets at t~0, then dummy
            # matmuls (p-state ramp) and a tiny tanh (absorbs the 1283ns
            # LoadActFuncSet before the real pipeline).
            jst = cst.tile([128, 128], BF16, tag="jst", name="jst")
            nc.vector.memset(jst[:], 0.25)
            jmv = cst.tile([128, 512], BF16, tag="jmv", name="jmv")
            nc.vector.memset(jmv[:], 0.25)
            jact = cst.tile([128, 16], F32, tag="jact", name="jact")
            nc.scalar.activation(jact[:], jst[:, 0:16], Tanh)

            # ---- small constants at the FRONT of the sync queue (land
            # before kt0; first kp matmul needs wkb)
            wts_sb = cst.tile([128, 384], BF16, tag="wts", name="wts_sb")
            nc.scalar.dma_start(out=wts_sb[:], in_=wts[:])
            aux_sb = cst.tile([128, AFREE], F32, tag="aux", name="aux_sb")
            nc.sync.dma_start(out=aux_sb[:], in_=aux[:])
            wkb = wts_sb[:, 0:128]
            wqb = wts_sb[:, 128:256]
            identb = wts_sb[:, 256:384]
            bkb = aux_sb[:, AOFF_BK:AOFF_BK + 1]
            bqb = aux_sb[:, AOFF_BQ:AOFF_BQ + 1]
            wcb = aux_sb[:, AOFF_WC:AOFF_WC + NJ]

            mk = mrp.tile([128, 4, S], I8, tag="mk", name="mk")

            kp = [kpp.tile([128, S], BF16, tag=f"kp{j}", name=f"kp{j}")
                  for j in range(NJ)]
            qp = [qpp.tile([128, SQ], BF16, tag=f"qp{j}", name=f"qp{j}")
                  for j in range(NJ)]

            warm = ppk.tile([128, S], F32, tag="pk", name="warm")
            for w in range(4):
                nc.tensor.matmul(warm[:, 0:512], jst[:], jmv[:],
                                 start=True, stop=True)

            # inline score tiles t=0,1 as per-half psum tiles (finer deps)
            psA = {(t, kh): psc.tile([128, 512], F32, tag="ps",
                                     name=f"psA_{t}_{kh}")
                   for t in range(2) for kh in range(2)}

            # ---- j-loop: input DMAs in consumption order on the sync
            # queue; unfused per-half kp proj + per-j qp proj (all [128,512]
            # single-bank psums, bufs=2 rotation); inline score matmuls for
            # t=0,1 emitted with a 3-j lag so the PE never head-of-line
            # blocks on the qp tanh/scale chain.
            pb2h0 = pb2p.tile([128, 512], F32, tag="p2", name="pb2h0")

            def emit_scores(jj):
                for t in range(2):
                    for kh in range(2):
                        nc.tensor.matmul(
                            psA[(t, kh)][:], qp[jj][:, ts(t, 128)],
                            kp[jj][:, ts(kh, 512)],
                            start=(jj == 0), stop=(jj == NJ - 1))
                nc.tensor.matmul(
                    pb2h0[:], qp[jj][:, ts(2, 128)], kp[jj][:, ts(0, 512)],
                    start=(jj == 0), stop=(jj == NJ - 1))

            for j in range(NJ):
                kt = kin.tile([128, S], BF16, tag="kt", name=f"kt{j}")
                if j == 0:
                    nc.sync.dma_start(out=kt[:, 0:512], in_=kT[0][:, 0:512])
                    nc.sync.dma_start(out=kt[:, 512:1024],
                                      in_=kT[0][:, 512:1024])
                else:
                    nc.sync.dma_start(out=kt[:], in_=kT[j])
                qt = qin.tile([128, SQ], BF16, tag="qt", name=f"qt{j}")
                nc.sync.dma_start(out=qt[:], in_=qT[j])
                pk = ppk.tile([128, S], F32, tag="pk", name=f"pk{j}")
                for kh in range(2):
                    nc.tensor.matmul(pk[:, ts(kh, 512)], wkb,
                                     kt[:, ts(kh, 512)],
                                     start=True, stop=True)
                nc.scalar.activation(kp[j][:], pk[:], Tanh, bias=bkb)

                pqj = ppq.tile([128, 512], F32, tag="pq", name=f"pq{j}")
                nc.tensor.matmul(pqj[:], wqb, qt[:], start=True, stop=True)
                tq = tqp.tile([128, SQ], BF16, tag="tq", name=f"tq{j}")
                nc.scalar.activation(tq[:], pqj[:], Tanh, bias=bqb)
                nc.vector.tensor_scalar_mul(qp[j][:], tq[:], wcb[:, j:j + 1])

                if j >= 3:
                    emit_scores(j - 3)
            for jj in (NJ - 3, NJ - 2, NJ - 1):
                emit_scores(jj)

            # mask arrives after the input stream; needed from the first tail
            nc.sync.dma_start(out=mk[:], in_=msk[:].rearrange("t p k -> p t k"))
            mka_sb = mrp.tile([128, 512], BF16, tag="mka", name="mka_sb")
            nc.sync.dma_start(out=mka_sb[:], in_=mska[:])

            def tail_halves(t, pst_halves):
                exs = softp.tile([128, S], F32, tag="ex", name=f"ex{t}")
                exm = obp.tile([128, S], F32, tag="exm", name=f"exm{t}")
                sh = [statp.tile([128, 1], F32, tag="ss", name=f"ss{t}_{h}")
                      for h in range(2)]
                for hh in range(2):
                    nc.scalar.activation(exs[:, ts(hh, 512)],
                                         pst_halves[hh][:], Exp)
                    nc.vector.scalar_tensor_tensor(
                        exm[:, ts(hh, 512)], exs[:, ts(hh, 512)], 1.0,
                        mk[:, t, ds(hh * 512, 512)],
                        op0=Byp, op1=Mult, accum_out=sh[hh][:])
                ssum = statp.tile([128, 1], F32, tag="ss", name=f"ssa{t}")
                nc.vector.tensor_tensor(ssum[:], sh[0][:], sh[1][:],
                                        op=mybir.AluOpType.add)
                rec = statp.tile([128, 1], F32, tag="rc", name=f"rc{t}")
                nc.vector.reciprocal(rec[:], ssum[:])
                ot = obp.tile([128, S], F16, tag="ot", name=f"ot{t}")
                for hh in range(2):
                    nc.vector.tensor_scalar_mul(
                        ot[:, ts(hh, 512)], exm[:, ts(hh, 512)], rec[:])
                nc.sync.dma_start(out=out[ts(t, 128), :], in_=ot[:])

            # ---- deferred t=2 kh1 on the ppq slot freed by the last tanh
            pb2h1 = ppq.tile([128, 512], F32, tag="pq", name="psB2_1")
            for jj in range(NJ):
                nc.tensor.matmul(
                    pb2h1[:], qp[jj][:, ts(2, 128)], kp[jj][:, ts(1, 512)],
                    start=(jj == 0), stop=(jj == NJ - 1))
            pb2 = [pb2h0, pb2h1]

            # inline-tile tails first: ready at loop end, and their psA
            # halves free up for t=3
            tail_halves(0, [psA[(0, 0)], psA[(0, 1)]])
            tail_halves(1, [psA[(1, 0)], psA[(1, 1)]])

            # ---- last tile t=3 on recycled psc half-slots, kh-major
            pb3 = [psc.tile([128, 512], F32, tag="ps", name=f"psB3_{kh}")
                   for kh in range(2)]
            for jj in range(NJ):
                nc.tensor.matmul(
                    pb3[0][:], qp[jj][:, ts(3, 128)], kp[jj][:, ts(0, 512)],
                    start=(jj == 0), stop=(jj == NJ - 1))
            # additive {0,-30000} mask initializes the kh1 psum through the
            # PE (identity stationary, start=True), so exp+accum_out yields
            # the masked row-sum directly - no DVE pass and no extra matmul
            # on the terminal chain
            nc.tensor.matmul(pb3[1][:], identb, mka_sb[:],
                             start=True, stop=False)
            for jj in range(NJ):
                nc.tensor.matmul(
                    pb3[1][:], qp[jj][:, ts(3, 128)], kp[jj][:, ts(1, 512)],
                    start=False, stop=(jj == NJ - 1))

            tail_halves(2, pb2)

            # t=3 tail: kh0 via the stt path, kh1 via exp+accum on the
            # pre-masked psum
            exs3 = softp.tile([128, S], F32, tag="ex", name="ex3")
            exm3 = obp.tile([128, S], F32, tag="exm", name="exm3")
            s3 = [statp.tile([128, 1], F32, tag="ss", name=f"ss3_{h}")
                  for h in range(2)]
            nc.scalar.activation(exs3[:, ts(0, 512)], pb3[0][:], Exp)
            nc.vector.scalar_tensor_tensor(
                exm3[:, ts(0, 512)], exs3[:, ts(0, 512)], 1.0,
                mk[:, 3, ds(0, 512)],
                op0=Byp, op1=Mult, accum_out=s3[0][:])
            nc.scalar.activation(exs3[:, ts(1, 512)], pb3[1][:], Exp,
                                 accum_out=s3[1][:])
            ssum3 = statp.tile([128, 1], F32, tag="ss", name="ssa3")
            nc.vector.tensor_tensor(ssum3[:], s3[0][:], s3[1][:],
                                    op=mybir.AluOpType.add)
            rec3 = statp.tile([128, 1], F32, tag="rc", name="rc3")
            nc.vector.reciprocal(rec3[:], ssum3[:])
            ot3 = obp.tile([128, S], F16, tag="ot", name="ot3")
            nc.vector.tensor_scalar_mul(
                ot3[:, ts(0, 512)], exm3[:, ts(0, 512)], rec3[:])
            nc.vector.tensor_scalar_mul(
                ot3[:, ts(1, 512)], exs3[:, ts(1, 512)], rec3[:])
            nc.sync.dma_start(out=out[ts(3, 128), :], in_=ot3[:])

    nc.compile()
    return nc


_NC = None


def _get_nc():
    global _NC
    if _NC is None:
        _NC = build_nc()
    return _NC


def make_in_maps(query, key, mask, Wq, bq, Wk, bk, Wc, bc):
    query = np.asarray(query, np.float32)
    key = np.asarray(key, np.float32)
    mask = np.asarray(mask)
    Wq = np.asarray(Wq, np.float32)
    Wk = np.asarray(Wk, np.float32)
    Wc = np.asarray(Wc, np.float32)
    bq = np.asarray(bq, np.float32)
    bk = np.asarray(bk, np.float32)

    def blockdiag(W):
        blk = np.zeros((128, 128), np.float32)
        blk[0:64, 0:64] = W.T
        blk[64:128, 64:128] = W.T
        return blk

    wts = np.zeros((128, 384), np.float32)
    wts[:, 0:128] = blockdiag(Wk)
    wts[:, 128:256] = blockdiag(Wq)
    wts[:, 256:384] = np.eye(128, dtype=np.float32)
    wts = wts.astype(NP_BF16)

    aux = np.zeros((128, AFREE), np.float32)
    aux[:, AOFF_BK] = np.tile(bk.reshape(-1), 2)
    aux[:, AOFF_BQ] = np.tile(bq.reshape(-1), 2)
    for j in range(NJ):
        aux[0:64, AOFF_WC + j] = Wc[0, 2 * j]
        aux[64:128, AOFF_WC + j] = Wc[0, 2 * j + 1]

    in_maps = []
    for c in range(NCORES):
        b, half = divmod(c, 2)
        s0 = half * SQ
        qh = query[b].reshape(H, S, DK)[:, s0:s0 + SQ, :]
        qTc = np.ascontiguousarray(
            qh.transpose(0, 2, 1)).reshape(NJ, 128, SQ).astype(NP_BF16)
        kh_ = key[b].reshape(H, S, DK)
        kTc = np.ascontiguousarray(
            kh_.transpose(0, 2, 1)).reshape(NJ, 128, S).astype(NP_BF16)

        mc = np.ascontiguousarray(
            mask[b, s0:s0 + SQ, :].reshape(4, 128, S)).astype(np.int8)
        mka = ((mask[b, s0 + 384:s0 + 512, 512:1024] == 0)
               .astype(np.float32) * -30000.0).astype(NP_BF16)
        in_maps.append({"qT": qTc, "kT": kTc, "msk": mc, "wts": wts,
                        "aux": aux, "mska": mka})
    return in_maps


def kernel(query, key, mask, Wq, bq, Wk, bk, Wc, bc):
    from concourse.bass_utils import run_bass_kernel_spmd

    nc = _get_nc()
    in_maps = make_in_maps(query, key, mask, Wq, bq, Wk, bk, Wc, bc)
    res = run_bass_kernel_spmd(nc, in_maps, list(range(NCORES)))
    full = np.empty((B, S, S), np.float32)
    for c in range(NCORES):
        b, half = divmod(c, 2)
        full[b, half * SQ:(half + 1) * SQ, :] = \
            res.results[c]["out"].astype(np.float32)
    return full


# revision 9
# speedup vs baseline: 1.0687x; 1.0086x over previous
"""Trainium2 Bass kernel for nn_MHParallelAttention (B=4,S=1024,H=16,DK=64).

Sharding: 8 cores = (batch) x (query-row half); each core owns output rows
[b, s0:s0+512, :] end-to-end, no collectives.

Algebra folds:
  * sum_h Wc_h*(q_h . k_h) == (concat_h Wc_h*q_h) . (concat_h k_h): scores +
    head-combine collapse to one [512,1024]@[1024,1024]^T matmul per core,
    PSUM-accumulated over 8 chunks of 128 features.
  * bc is softmax-shift-invariant -> dropped.
  * block-diag [[W,0],[0,W]] 128x128 projection weights process a head PAIR
    per matmul.
  * softmax without max-subtraction (|logit| <= ~6 here); 0/1 int8 mask
    applied multiplicatively AFTER exp, fused with the row-sum in one DVE
    scalar_tensor_tensor pass.

Schedule (cost-model timeline 31.5us vs 39.9us for the f32r baseline):
  * all matmuls bf16: same 1 cycle/row as f32r on the PE, but half the DMA
    bytes - the DMA engines are a single serial ~17us-at-f32 resource.
  * j-loop: kt_j/qt_j DMAs in consumption order; kp proj as 2 matmuls
    into one [128,1024] 2-bank psum + ONE fused tanh (the per-j qp tanh
    fills the single-buffered rotation window, keeping ACT gapless); qp
    proj per j + tanh + DVE wc-scale (bf16 4x mode).
    Inline score accumulation for row-tiles t=0,1 into four single-bank
    half-psums, emitted with a 2-j lag so the PE never head-of-line blocks
    on the qp tanh/scale chain.
  * t=2 kh0 accumulates inline on the bank freed by making the qp-proj
    psum single-buffered (that chain has 1.8us of ACT slack per step). Deferred after the loop: t2-kh1 on the ppq
    slot, t3 halves on psc slots freed by t=0's exps - per-half psum tiles
    so each half's exp chains off its own 8 matmuls.
  * tails: exp (ACT) -> masked-sum stt (DVE, accum_out) -> reciprocal ->
    per-half scale -> ONE fp16 [128,1024] out DMA per tile on the sync
    queue (HWDGE descriptor-gen is a serial 625ns/DMA device - fewer,
    bigger DMAs; host upcasts fp16->f32, quantization ~5e-4 << 2e-2 tol).
  * terminal chains (t=2 and t=3, kh1): the 0/-30000 additive mask is
    folded into the psum by an identity-stationary matmul (start=True
    initializer), so exp+accum_out yields the masked row-sum directly -
    no DVE masked-sum pass on the critical chains; each tile then takes
    ONE fused [128,1024] scale before its single out-DMA.
  * warmup: DVE memsets + 4 dummy matmuls ramp the PE p-state before real
    work; a tiny tanh absorbs the 1283ns LoadActFuncSet; wts rides the
    scalar-queue DGE in parallel with the sync queue's kt stream.

Host-side prep is layout/dtype-only; all FLOPs run on device.
"""

import os
import sys

import numpy as np

for _p in ("/opt/trn_rl_repo", "/root/.axon_site/_ro/trn_rl_repo"):
    if os.path.isdir(_p) and _p not in sys.path:
        sys.path.insert(0, _p)

import concourse.bass as bass
import concourse.mybir as mybir
import concourse.tile as tile
from concourse import bacc
from concourse.bass import ds, ts

H, DK = 16, 64
B, S = 4, 1024
SQ = 512
NCORES = 8
NJ = 8

F32 = mybir.dt.float32
F16 = mybir.dt.float16
BF16 = mybir.dt.bfloat16
I8 = mybir.dt.int8
NP_BF16 = mybir.dt.np(BF16)

# aux f32 layout along free dim: bk | bq | wc[8]
AOFF_BK, AOFF_BQ, AOFF_WC = 0, 1, 2
AFREE = 10

Tanh = mybir.ActivationFunctionType.Tanh
Exp = mybir.ActivationFunctionType.Exp
Byp = mybir.AluOpType.bypass
Mult = mybir.AluOpType.mult


def build_nc():
    nc = bacc.Bacc(None, target_bir_lowering=False, debug=False)

    qT = nc.dram_tensor("qT", [NJ, 128, SQ], BF16, kind="ExternalInput")
    kT = nc.dram_tensor("kT", [NJ, 128, S], BF16, kind="ExternalInput")
    msk = nc.dram_tensor("msk", [4, 128, S], I8, kind="ExternalInput")
    wts = nc.dram_tensor("wts", [128, 384], BF16, kind="ExternalInput")
    mska = nc.dram_tensor("mska", [128, 1024], BF16, kind="ExternalInput")
    aux = nc.dram_tensor("aux", [128, AFREE], F32, kind="ExternalInput")
    out = nc.dram_tensor("out", [SQ, S], F16, kind="ExternalOutput")

    with tile.TileContext(nc) as tc:
        with (
            tc.tile_pool(name="const", bufs=1) as cst,
            tc.tile_pool(name="kin", bufs=3) as kin,
            tc.tile_pool(name="qin", bufs=3) as qin,
            tc.tile_pool(name="kpp", bufs=1) as kpp,
            tc.tile_pool(name="qpp", bufs=1) as qpp,
            tc.tile_pool(name="tqp", bufs=3) as tqp,
            tc.tile_pool(name="mrow", bufs=1) as mrp,
            tc.tile_pool(name="soft", bufs=4) as softp,
            tc.tile_pool(name="stat", bufs=8) as statp,
            tc.tile_pool(name="obuf", bufs=4) as obp,
            tc.tile_pool(name="ppk", bufs=1, space="PSUM") as ppk,
            tc.tile_pool(name="ppq", bufs=1, space="PSUM") as ppq,
            tc.tile_pool(name="pb2p", bufs=1, space="PSUM") as pb2p,
            tc.tile_pool(name="psc", bufs=4, space="PSUM") as psc,
        ):
            # ---- warmup junk first: DVE memsets at t~0, then dummy
            # matmuls (p-state ramp) and a tiny tanh (absorbs the 1283ns
            # LoadActFuncSet before the real pipeline).
            jst = cst.tile([128, 128], BF16, tag="jst", name="jst")
            nc.vector.memset(jst[:], 0.25)
            jmv = cst.tile([128, 512], BF16, tag="jmv", name="jmv")
            nc.vector.memset(jmv[:], 0.25)
            jact = cst.tile([128, 16], F32, tag="jact", name="jact")
            nc.scalar.activation(jact[:], jst[:, 0:16], Tanh)

            # ---- small constants at the FRONT of the sync queue (land
            # before kt0; first kp matmul needs wkb)
            wts_sb = cst.tile([128, 384], BF16, tag="wts", name="wts_sb")
            nc.scalar.dma_start(out=wts_sb[:], in_=wts[:])
            aux_sb = cst.tile([128, AFREE], F32, tag="aux", name="aux_sb")
            nc.sync.dma_start(out=aux_sb[:], in_=aux[:])
            wkb = wts_sb[:, 0:128]
            wqb = wts_sb[:, 128:256]
            identb = wts_sb[:, 256:384]
            bkb = aux_sb[:, AOFF_BK:AOFF_BK + 1]
            bqb = aux_sb[:, AOFF_BQ:AOFF_BQ + 1]
            wcb = aux_sb[:, AOFF_WC:AOFF_WC + NJ]

            mk = mrp.tile([128, 4, S], I8, tag="mk", name="mk")

            kp = [kpp.tile([128, S], BF16, tag=f"kp{j}", name=f"kp{j}")
                  for j in range(NJ)]
            qp = [qpp.tile([128, SQ], BF16, tag=f"qp{j}", name=f"qp{j}")
                  for j in range(NJ)]

            warm = ppk.tile([128, S], F32, tag="pk", name="warm")
            for w in range(3):
                nc.tensor.matmul(warm[:, 0:512], jst[:], jmv[:],
                                 start=True, stop=True)

            # inline score tiles t=0,1 as per-half psum tiles (finer deps)
            psA = {(t, kh): psc.tile([128, 512], F32, tag="ps",
                                     name=f"psA_{t}_{kh}")
                   for t in range(2) for kh in range(2)}

            # ---- j-loop: input DMAs in consumption order on the sync
            # queue; unfused per-half kp proj + per-j qp proj (all [128,512]
            # single-bank psums, bufs=2 rotation); inline score matmuls for
            # t=0,1 emitted with a 3-j lag so the PE never head-of-line
            # blocks on the qp tanh/scale chain.
            pb2h0 = pb2p.tile([128, 512], F32, tag="p2", name="pb2h0")

            def emit_scores(jj):
                for t in range(2):
                    for kh in range(2):
                        nc.tensor.matmul(
                            psA[(t, kh)][:], qp[jj][:, ts(t, 128)],
                            kp[jj][:, ts(kh, 512)],
                            start=(jj == 0), stop=(jj == NJ - 1))
                nc.tensor.matmul(
                    pb2h0[:], qp[jj][:, ts(2, 128)], kp[jj][:, ts(0, 512)],
                    start=(jj == 0), stop=(jj == NJ - 1))

            for j in range(NJ):
                kt = kin.tile([128, S], BF16, tag="kt", name=f"kt{j}")
                if j == 0:
                    nc.sync.dma_start(out=kt[:, 0:512], in_=kT[0][:, 0:512])
                    nc.sync.dma_start(out=kt[:, 512:1024],
                                      in_=kT[0][:, 512:1024])
                else:
                    nc.sync.dma_start(out=kt[:], in_=kT[j])
                qt = qin.tile([128, SQ], BF16, tag="qt", name=f"qt{j}")
                nc.sync.dma_start(out=qt[:], in_=qT[j])
                pk = ppk.tile([128, S], F32, tag="pk", name=f"pk{j}")
                for kh in range(2):
                    nc.tensor.matmul(pk[:, ts(kh, 512)], wkb,
                                     kt[:, ts(kh, 512)],
                                     start=True, stop=True)
                nc.scalar.activation(kp[j][:], pk[:], Tanh, bias=bkb)

                pqj = ppq.tile([128, 512], F32, tag="pq", name=f"pq{j}")
                nc.tensor.matmul(pqj[:], wqb, qt[:], start=True, stop=True)
                tq = tqp.tile([128, SQ], BF16, tag="tq", name=f"tq{j}")
                nc.scalar.activation(tq[:], pqj[:], Tanh, bias=bqb)
                nc.vector.tensor_scalar_mul(qp[j][:], tq[:], wcb[:, j:j + 1])

                if j >= 3:
                    emit_scores(j - 3)
            for jj in (NJ - 3, NJ - 2, NJ - 1):
                emit_scores(jj)

            # mask arrives after the input stream; needed from the first tail
            nc.sync.dma_start(out=mk[:], in_=msk[:].rearrange("t p k -> p t k"))
            mka_sb = mrp.tile([128, 1024], BF16, tag="mka", name="mka_sb")
            nc.sync.dma_start(out=mka_sb[:], in_=mska[:])

            def tail_halves(t, pst_halves):
                exs = softp.tile([128, S], F32, tag="ex", name=f"ex{t}")
                exm = obp.tile([128, S], F32, tag="exm", name=f"exm{t}")
                sh = [statp.tile([128, 1], F32, tag="ss", name=f"ss{t}_{h}")
                      for h in range(2)]
                for hh in range(2):
                    nc.scalar.activation(exs[:, ts(hh, 512)],
                                         pst_halves[hh][:], Exp)
                    nc.vector.scalar_tensor_tensor(
                        exm[:, ts(hh, 512)], exs[:, ts(hh, 512)], 1.0,
                        mk[:, t, ds(hh * 512, 512)],
                        op0=Byp, op1=Mult, accum_out=sh[hh][:])
                ssum = statp.tile([128, 1], F32, tag="ss", name=f"ssa{t}")
                nc.vector.tensor_tensor(ssum[:], sh[0][:], sh[1][:],
                                        op=mybir.AluOpType.add)
                rec = statp.tile([128, 1], F32, tag="rc", name=f"rc{t}")
                nc.vector.reciprocal(rec[:], ssum[:])
                ot = obp.tile([128, S], F16, tag="ot", name=f"ot{t}")
                nc.vector.tensor_scalar_mul(ot[:], exm[:], rec[:])
                nc.sync.dma_start(out=out[ts(t, 128), :], in_=ot[:])

            # ---- deferred t=2 kh1 on the ppq slot freed by the last tanh;
            # additive mask initializes the psum (exp+accum -> masked sum)
            pb2h1 = ppq.tile([128, 512], F32, tag="pq", name="psB2_1")
            nc.tensor.matmul(pb2h1[:], identb, mka_sb[:, 0:512],
                             start=True, stop=False)
            for jj in range(NJ):
                nc.tensor.matmul(
                    pb2h1[:], qp[jj][:, ts(2, 128)], kp[jj][:, ts(1, 512)],
                    start=False, stop=(jj == NJ - 1))
            pb2 = [pb2h0, pb2h1]

            # inline-tile tails first: ready at loop end, and their psA
            # halves free up for t=3
            tail_halves(0, [psA[(0, 0)], psA[(0, 1)]])
            tail_halves(1, [psA[(1, 0)], psA[(1, 1)]])

            # ---- last tile t=3 on recycled psc half-slots, kh-major
            pb3 = [psc.tile([128, 512], F32, tag="ps", name=f"psB3_{kh}")
                   for kh in range(2)]
            for jj in range(NJ):
                nc.tensor.matmul(
                    pb3[0][:], qp[jj][:, ts(3, 128)], kp[jj][:, ts(0, 512)],
                    start=(jj == 0), stop=(jj == NJ - 1))
            # additive {0,-30000} mask initializes the kh1 psum through the
            # PE (identity stationary, start=True), so exp+accum_out yields
            # the masked row-sum directly - no DVE pass and no extra matmul
            # on the terminal chain
            nc.tensor.matmul(pb3[1][:], identb, mka_sb[:, 512:1024],
                             start=True, stop=False)
            for jj in range(NJ):
                nc.tensor.matmul(
                    pb3[1][:], qp[jj][:, ts(3, 128)], kp[jj][:, ts(1, 512)],
                    start=False, stop=(jj == NJ - 1))

            # t=2 tail: kh0 via stt, kh1 via exp+accum on the pre-masked psum
            exs2 = softp.tile([128, S], F32, tag="ex", name="ex2")
            exm2 = obp.tile([128, S], F32, tag="exm", name="exm2")
            s2 = [statp.tile([128, 1], F32, tag="ss", name=f"ss2_{h}")
                  for h in range(2)]
            nc.scalar.activation(exs2[:, ts(0, 512)], pb2[0][:], Exp)
            nc.vector.scalar_tensor_tensor(
                exm2[:, ts(0, 512)], exs2[:, ts(0, 512)], 1.0,
                mk[:, 2, ds(0, 512)],
                op0=Byp, op1=Mult, accum_out=s2[0][:])
            nc.scalar.activation(exm2[:, ts(1, 512)], pb2[1][:], Exp,
                                 accum_out=s2[1][:])
            ssum2 = statp.tile([128, 1], F32, tag="ss", name="ssa2")
            nc.vector.tensor_tensor(ssum2[:], s2[0][:], s2[1][:],
                                    op=mybir.AluOpType.add)
            rec2 = statp.tile([128, 1], F32, tag="rc", name="rc2")
            nc.vector.reciprocal(rec2[:], ssum2[:])
            ot2 = obp.tile([128, S], F16, tag="ot", name="ot2")
            nc.vector.tensor_scalar_mul(ot2[:], exm2[:], rec2[:])
            nc.sync.dma_start(out=out[ts(2, 128), :], in_=ot2[:])

            # t=3 tail: kh0 via the stt path, kh1 via exp+accum on the
            # pre-masked psum
            exs3 = softp.tile([128, S], F32, tag="ex", name="ex3")
            exm3 = obp.tile([128, S], F32, tag="exm", name="exm3")
            s3 = [statp.tile([128, 1], F32, tag="ss", name=f"ss3_{h}")
                  for h in range(2)]
            nc.scalar.activation(exm3[:, ts(0, 512)], pb3[0][:], Exp)
            nc.vector.scalar_tensor_tensor(
                exs3[:, ts(0, 512)], exm3[:, ts(0, 512)], 1.0,
                mk[:, 3, ds(0, 512)],
                op0=Byp, op1=Mult, accum_out=s3[0][:])
            nc.scalar.activation(exs3[:, ts(1, 512)], pb3[1][:], Exp,
                                 accum_out=s3[1][:])
            ssum3 = statp.tile([128, 1], F32, tag="ss", name="ssa3")
            nc.vector.tensor_tensor(ssum3[:], s3[0][:], s3[1][:],
                                    op=mybir.AluOpType.add)
            rec3 = statp.tile([128, 1], F32, tag="rc", name="rc3")
            nc.vector.reciprocal(rec3[:], ssum3[:])
            ot3 = obp.tile([128, S], F16, tag="ot", name="ot3")
            nc.vector.tensor_scalar_mul(ot3[:], exs3[:], rec3[:])
            nc.sync.dma_start(out=out[ts(3, 128), :], in_=ot3[:])

    nc.compile()
    return nc


_NC = None


def _get_nc():
    global _NC
    if _NC is None:
        _NC = build_nc()
    return _NC


def make_in_maps(query, key, mask, Wq, bq, Wk, bk, Wc, bc):
    query = np.asarray(query, np.float32)
    key = np.asarray(key, np.float32)
    mask = np.asarray(mask)
    Wq = np.asarray(Wq, np.float32)
    Wk = np.asarray(Wk, np.float32)
    Wc = np.asarray(Wc, np.float32)
    bq = np.asarray(bq, np.float32)
    bk = np.asarray(bk, np.float32)

    def blockdiag(W):
        blk = np.zeros((128, 128), np.float32)
        blk[0:64, 0:64] = W.T
        blk[64:128, 64:128] = W.T
        return blk

    wts = np.zeros((128, 384), np.float32)
    wts[:, 0:128] = blockdiag(Wk)
    wts[:, 128:256] = blockdiag(Wq)
    wts[:, 256:384] = np.eye(128, dtype=np.float32)
    wts = wts.astype(NP_BF16)

    aux = np.zeros((128, AFREE), np.float32)
    aux[:, AOFF_BK] = np.tile(bk.reshape(-1), 2)
    aux[:, AOFF_BQ] = np.tile(bq.reshape(-1), 2)
    for j in range(NJ):
        aux[0:64, AOFF_WC + j] = Wc[0, 2 * j]
        aux[64:128, AOFF_WC + j] = Wc[0, 2 * j + 1]

    in_maps = []
    for c in range(NCORES):
        b, half = divmod(c, 2)
        s0 = half * SQ
        qh = query[b].reshape(H, S, DK)[:, s0:s0 + SQ, :]
        qTc = np.ascontiguousarray(
            qh.transpose(0, 2, 1)).reshape(NJ, 128, SQ).astype(NP_BF16)
        kh_ = key[b].reshape(H, S, DK)
        kTc = np.ascontiguousarray(
            kh_.transpose(0, 2, 1)).reshape(NJ, 128, S).astype(NP_BF16)

        mc = np.ascontiguousarray(
            mask[b, s0:s0 + SQ, :].reshape(4, 128, S)).astype(np.int8)
        mka = np.concatenate([
            ((mask[b, s0 + 256:s0 + 384, 512:1024] == 0)
             .astype(np.float32) * -30000.0),
            ((mask[b, s0 + 384:s0 + 512, 512:1024] == 0)
             .astype(np.float32) * -30000.0)], axis=1).astype(NP_BF16)
        in_maps.append({"qT": qTc, "kT": kTc, "msk": mc, "wts": wts,
                        "aux": aux, "mska": mka})
    return in_maps


def kernel(query, key, mask, Wq, bq, Wk, bk, Wc, bc):
    from concourse.bass_utils import run_bass_kernel_spmd

    nc = _get_nc()
    in_maps = make_in_maps(query, key, mask, Wq, bq, Wk, bk, Wc, bc)
    res = run_bass_kernel_spmd(nc, in_maps, list(range(NCORES)))
    full = np.empty((B, S, S), np.float32)
    for c in range(NCORES):
        b, half = divmod(c, 2)
        full[b, half * SQ:(half + 1) * SQ, :] = \
            res.results[c]["out"].astype(np.float32)
    return full


# revision 10
# speedup vs baseline: 1.0691x; 1.0004x over previous
"""Trainium2 Bass kernel for nn_MHParallelAttention (B=4,S=1024,H=16,DK=64).

Sharding: 8 cores = (batch) x (query-row half); each core owns output rows
[b, s0:s0+512, :] end-to-end, no collectives.

Algebra folds:
  * sum_h Wc_h*(q_h . k_h) == (concat_h Wc_h*q_h) . (concat_h k_h): scores +
    head-combine collapse to one [512,1024]@[1024,1024]^T matmul per core,
    PSUM-accumulated over 8 chunks of 128 features.
  * bc is softmax-shift-invariant -> dropped.
  * block-diag [[W,0],[0,W]] 128x128 projection weights process a head PAIR
    per matmul.
  * softmax without max-subtraction (|logit| <= ~6 here); 0/1 int8 mask
    applied multiplicatively AFTER exp, fused with the row-sum in one DVE
    scalar_tensor_tensor pass.

Schedule (cost-model timeline 31.2us vs 39.9us for the f32r baseline):
  * all matmuls bf16: same 1 cycle/row as f32r on the PE, but half the DMA
    bytes - the DMA engines are a single serial ~17us-at-f32 resource.
  * j-loop: kt_j/qt_j DMAs in consumption order; kp proj as 2 matmuls
    into one [128,1024] 2-bank psum + ONE fused tanh (the per-j qp tanh
    fills the single-buffered rotation window, keeping ACT gapless); qp
    proj per j + tanh + DVE wc-scale (bf16 4x mode).
    Inline score accumulation for row-tiles t=0,1 into four single-bank
    half-psums, emitted with a 2-j lag so the PE never head-of-line blocks
    on the qp tanh/scale chain.
  * t=2 kh0 accumulates inline on the bank freed by making the qp-proj
    psum single-buffered (that chain has 1.8us of ACT slack per step). Deferred after the loop: t2-kh1 on the ppq
    slot, t3 halves on psc slots freed by t=0's exps - per-half psum tiles
    so each half's exp chains off its own 8 matmuls.
  * tails: exp (ACT) -> masked-sum stt (DVE, accum_out) -> reciprocal ->
    per-half scale -> ONE fp16 [128,1024] out DMA per tile on the sync
    queue (HWDGE descriptor-gen is a serial 625ns/DMA device - fewer,
    bigger DMAs; host upcasts fp16->f32, quantization ~5e-4 << 2e-2 tol).
  * terminal chains (t=2 and t=3, kh1): the 0/-30000 additive mask is
    folded into the psum by an identity-stationary matmul (start=True
    initializer), so exp+accum_out yields the masked row-sum directly -
    no DVE masked-sum pass on the critical chains; exp intermediates are
    bf16 so each tile's single fused [128,1024] scale (bf16 -> fp16) runs
    in the DVE 4x mode before its single out-DMA.
  * warmup: DVE memsets + 4 dummy matmuls ramp the PE p-state before real
    work; a tiny tanh absorbs the 1283ns LoadActFuncSet; wts rides the
    scalar-queue DGE in parallel with the sync queue's kt stream.

Host-side prep is layout/dtype-only; all FLOPs run on device.
"""

import os
import sys

import numpy as np

for _p in ("/opt/trn_rl_repo", "/root/.axon_site/_ro/trn_rl_repo"):
    if os.path.isdir(_p) and _p not in sys.path:
        sys.path.insert(0, _p)

import concourse.bass as bass
import concourse.mybir as mybir
import concourse.tile as tile
from concourse import bacc
from concourse.bass import ds, ts

H, DK = 16, 64
B, S = 4, 1024
SQ = 512
NCORES = 8
NJ = 8

F32 = mybir.dt.float32
F16 = mybir.dt.float16
BF16 = mybir.dt.bfloat16
I8 = mybir.dt.int8
NP_BF16 = mybir.dt.np(BF16)

# aux f32 layout along free dim: bk | bq | wc[8]
AOFF_BK, AOFF_BQ, AOFF_WC = 0, 1, 2
AFREE = 10

Tanh = mybir.ActivationFunctionType.Tanh
Exp = mybir.ActivationFunctionType.Exp
Byp = mybir.AluOpType.bypass
Mult = mybir.AluOpType.mult


def build_nc():
    nc = bacc.Bacc(None, target_bir_lowering=False, debug=False)

    qT = nc.dram_tensor("qT", [NJ, 128, SQ], BF16, kind="ExternalInput")
    kT = nc.dram_tensor("kT", [NJ, 128, S], BF16, kind="ExternalInput")
    msk = nc.dram_tensor("msk", [4, 128, S], I8, kind="ExternalInput")
    wts = nc.dram_tensor("wts", [128, 384], BF16, kind="ExternalInput")
    mska = nc.dram_tensor("mska", [128, 1024], BF16, kind="ExternalInput")
    aux = nc.dram_tensor("aux", [128, AFREE], F32, kind="ExternalInput")
    out = nc.dram_tensor("out", [SQ, S], F16, kind="ExternalOutput")

    with tile.TileContext(nc) as tc:
        with (
            tc.tile_pool(name="const", bufs=1) as cst,
            tc.tile_pool(name="kin", bufs=3) as kin,
            tc.tile_pool(name="qin", bufs=3) as qin,
            tc.tile_pool(name="kpp", bufs=1) as kpp,
            tc.tile_pool(name="qpp", bufs=1) as qpp,
            tc.tile_pool(name="tqp", bufs=3) as tqp,
            tc.tile_pool(name="mrow", bufs=1) as mrp,
            tc.tile_pool(name="soft", bufs=4) as softp,
            tc.tile_pool(name="stat", bufs=8) as statp,
            tc.tile_pool(name="obuf", bufs=4) as obp,
            tc.tile_pool(name="ppk", bufs=1, space="PSUM") as ppk,
            tc.tile_pool(name="ppq", bufs=1, space="PSUM") as ppq,
            tc.tile_pool(name="pb2p", bufs=1, space="PSUM") as pb2p,
            tc.tile_pool(name="psc", bufs=4, space="PSUM") as psc,
        ):
            # ---- warmup junk first: DVE memsets at t~0, then dummy
            # matmuls (p-state ramp) and a tiny tanh (absorbs the 1283ns
            # LoadActFuncSet before the real pipeline).
            jst = cst.tile([128, 128], BF16, tag="jst", name="jst")
            nc.vector.memset(jst[:], 0.25)
            jmv = cst.tile([128, 512], BF16, tag="jmv", name="jmv")
            nc.vector.memset(jmv[:], 0.25)
            jact = cst.tile([128, 16], F32, tag="jact", name="jact")
            nc.scalar.activation(jact[:], jst[:, 0:16], Tanh)

            # ---- small constants at the FRONT of the sync queue (land
            # before kt0; first kp matmul needs wkb)
            wts_sb = cst.tile([128, 384], BF16, tag="wts", name="wts_sb")
            nc.scalar.dma_start(out=wts_sb[:], in_=wts[:])
            aux_sb = cst.tile([128, AFREE], F32, tag="aux", name="aux_sb")
            nc.sync.dma_start(out=aux_sb[:], in_=aux[:])
            wkb = wts_sb[:, 0:128]
            wqb = wts_sb[:, 128:256]
            identb = wts_sb[:, 256:384]
            bkb = aux_sb[:, AOFF_BK:AOFF_BK + 1]
            bqb = aux_sb[:, AOFF_BQ:AOFF_BQ + 1]
            wcb = aux_sb[:, AOFF_WC:AOFF_WC + NJ]

            mk = mrp.tile([128, 4, S], I8, tag="mk", name="mk")

            kp = [kpp.tile([128, S], BF16, tag=f"kp{j}", name=f"kp{j}")
                  for j in range(NJ)]
            qp = [qpp.tile([128, SQ], BF16, tag=f"qp{j}", name=f"qp{j}")
                  for j in range(NJ)]

            warm = ppk.tile([128, S], F32, tag="pk", name="warm")
            for w in range(3):
                nc.tensor.matmul(warm[:, 0:512], jst[:], jmv[:],
                                 start=True, stop=True)

            # inline score tiles t=0,1 as per-half psum tiles (finer deps)
            psA = {(t, kh): psc.tile([128, 512], F32, tag="ps",
                                     name=f"psA_{t}_{kh}")
                   for t in range(2) for kh in range(2)}

            # ---- j-loop: input DMAs in consumption order on the sync
            # queue; unfused per-half kp proj + per-j qp proj (all [128,512]
            # single-bank psums, bufs=2 rotation); inline score matmuls for
            # t=0,1 emitted with a 3-j lag so the PE never head-of-line
            # blocks on the qp tanh/scale chain.
            pb2h0 = pb2p.tile([128, 512], F32, tag="p2", name="pb2h0")

            def emit_scores(jj):
                for t in range(2):
                    for kh in range(2):
                        nc.tensor.matmul(
                            psA[(t, kh)][:], qp[jj][:, ts(t, 128)],
                            kp[jj][:, ts(kh, 512)],
                            start=(jj == 0), stop=(jj == NJ - 1))
                nc.tensor.matmul(
                    pb2h0[:], qp[jj][:, ts(2, 128)], kp[jj][:, ts(0, 512)],
                    start=(jj == 0), stop=(jj == NJ - 1))

            for j in range(NJ):
                kt = kin.tile([128, S], BF16, tag="kt", name=f"kt{j}")
                if j == 0:
                    nc.sync.dma_start(out=kt[:, 0:512], in_=kT[0][:, 0:512])
                    nc.sync.dma_start(out=kt[:, 512:1024],
                                      in_=kT[0][:, 512:1024])
                else:
                    nc.sync.dma_start(out=kt[:], in_=kT[j])
                qt = qin.tile([128, SQ], BF16, tag="qt", name=f"qt{j}")
                nc.sync.dma_start(out=qt[:], in_=qT[j])
                pk = ppk.tile([128, S], F32, tag="pk", name=f"pk{j}")
                for kh in range(2):
                    nc.tensor.matmul(pk[:, ts(kh, 512)], wkb,
                                     kt[:, ts(kh, 512)],
                                     start=True, stop=True)
                nc.scalar.activation(kp[j][:], pk[:], Tanh, bias=bkb)

                pqj = ppq.tile([128, 512], F32, tag="pq", name=f"pq{j}")
                nc.tensor.matmul(pqj[:], wqb, qt[:], start=True, stop=True)
                tq = tqp.tile([128, SQ], BF16, tag="tq", name=f"tq{j}")
                nc.scalar.activation(tq[:], pqj[:], Tanh, bias=bqb)
                nc.vector.tensor_scalar_mul(qp[j][:], tq[:], wcb[:, j:j + 1])

                if j >= 3:
                    emit_scores(j - 3)
            for jj in (NJ - 3, NJ - 2, NJ - 1):
                emit_scores(jj)

            # mask arrives after the input stream; needed from the first tail
            nc.sync.dma_start(out=mk[:], in_=msk[:].rearrange("t p k -> p t k"))
            mka_sb = mrp.tile([128, 1024], BF16, tag="mka", name="mka_sb")
            nc.sync.dma_start(out=mka_sb[:], in_=mska[:])

            def tail_halves(t, pst_halves):
                exs = softp.tile([128, S], BF16, tag="ex", name=f"ex{t}")
                exm = obp.tile([128, S], BF16, tag="exm", name=f"exm{t}")
                sh = [statp.tile([128, 1], F32, tag="ss", name=f"ss{t}_{h}")
                      for h in range(2)]
                for hh in range(2):
                    nc.scalar.activation(exs[:, ts(hh, 512)],
                                         pst_halves[hh][:], Exp)
                    nc.vector.scalar_tensor_tensor(
                        exm[:, ts(hh, 512)], exs[:, ts(hh, 512)], 1.0,
                        mk[:, t, ds(hh * 512, 512)],
                        op0=Byp, op1=Mult, accum_out=sh[hh][:])
                ssum = statp.tile([128, 1], F32, tag="ss", name=f"ssa{t}")
                nc.vector.tensor_tensor(ssum[:], sh[0][:], sh[1][:],
                                        op=mybir.AluOpType.add)
                rec = statp.tile([128, 1], F32, tag="rc", name=f"rc{t}")
                nc.vector.reciprocal(rec[:], ssum[:])
                ot = obp.tile([128, S], F16, tag="ot", name=f"ot{t}")
                nc.vector.tensor_scalar_mul(ot[:], exm[:], rec[:])
                nc.sync.dma_start(out=out[ts(t, 128), :], in_=ot[:])

            # ---- deferred t=2 kh1 on the ppq slot freed by the last tanh;
            # additive mask initializes the psum (exp+accum -> masked sum)
            pb2h1 = ppq.tile([128, 512], F32, tag="pq", name="psB2_1")
            nc.tensor.matmul(pb2h1[:], identb, mka_sb[:, 0:512],
                             start=True, stop=False)
            for jj in range(NJ):
                nc.tensor.matmul(
                    pb2h1[:], qp[jj][:, ts(2, 128)], kp[jj][:, ts(1, 512)],
                    start=False, stop=(jj == NJ - 1))
            pb2 = [pb2h0, pb2h1]

            # inline-tile tails first: ready at loop end, and their psA
            # halves free up for t=3
            tail_halves(0, [psA[(0, 0)], psA[(0, 1)]])
            tail_halves(1, [psA[(1, 0)], psA[(1, 1)]])

            # ---- last tile t=3 on recycled psc half-slots, kh-major
            pb3 = [psc.tile([128, 512], F32, tag="ps", name=f"psB3_{kh}")
                   for kh in range(2)]
            for jj in range(NJ):
                nc.tensor.matmul(
                    pb3[0][:], qp[jj][:, ts(3, 128)], kp[jj][:, ts(0, 512)],
                    start=(jj == 0), stop=(jj == NJ - 1))
            # additive {0,-30000} mask initializes the kh1 psum through the
            # PE (identity stationary, start=True), so exp+accum_out yields
            # the masked row-sum directly - no DVE pass and no extra matmul
            # on the terminal chain
            nc.tensor.matmul(pb3[1][:], identb, mka_sb[:, 512:1024],
                             start=True, stop=False)
            for jj in range(NJ):
                nc.tensor.matmul(
                    pb3[1][:], qp[jj][:, ts(3, 128)], kp[jj][:, ts(1, 512)],
                    start=False, stop=(jj == NJ - 1))

            # t=2 tail: kh0 via stt, kh1 via exp+accum on the pre-masked psum
            exs2 = softp.tile([128, S], BF16, tag="ex", name="ex2")
            exm2 = obp.tile([128, S], BF16, tag="exm", name="exm2")
            s2 = [statp.tile([128, 1], F32, tag="ss", name=f"ss2_{h}")
                  for h in range(2)]
            nc.scalar.activation(exs2[:, ts(0, 512)], pb2[0][:], Exp)
            nc.vector.scalar_tensor_tensor(
                exm2[:, ts(0, 512)], exs2[:, ts(0, 512)], 1.0,
                mk[:, 2, ds(0, 512)],
                op0=Byp, op1=Mult, accum_out=s2[0][:])
            nc.scalar.activation(exm2[:, ts(1, 512)], pb2[1][:], Exp,
                                 accum_out=s2[1][:])
            ssum2 = statp.tile([128, 1], F32, tag="ss", name="ssa2")
            nc.vector.tensor_tensor(ssum2[:], s2[0][:], s2[1][:],
                                    op=mybir.AluOpType.add)
            rec2 = statp.tile([128, 1], F32, tag="rc", name="rc2")
            nc.vector.reciprocal(rec2[:], ssum2[:])
            ot2 = obp.tile([128, S], F16, tag="ot", name="ot2")
            nc.vector.tensor_scalar_mul(ot2[:], exm2[:], rec2[:])
            nc.sync.dma_start(out=out[ts(2, 128), :], in_=ot2[:])

            # t=3 tail: kh0 via the stt path, kh1 via exp+accum on the
            # pre-masked psum
            exs3 = softp.tile([128, S], BF16, tag="ex", name="ex3")
            exm3 = obp.tile([128, S], BF16, tag="exm", name="exm3")
            s3 = [statp.tile([128, 1], F32, tag="ss", name=f"ss3_{h}")
                  for h in range(2)]
            nc.scalar.activation(exm3[:, ts(0, 512)], pb3[0][:], Exp)
            nc.vector.scalar_tensor_tensor(
                exs3[:, ts(0, 512)], exm3[:, ts(0, 512)], 1.0,
                mk[:, 3, ds(0, 512)],
                op0=Byp, op1=Mult, accum_out=s3[0][:])
            nc.scalar.activation(exs3[:, ts(1, 512)], pb3[1][:], Exp,
                                 accum_out=s3[1][:])
            ssum3 = statp.tile([128, 1], F32, tag="ss", name="ssa3")
            nc.vector.tensor_tensor(ssum3[:], s3[0][:], s3[1][:],
                                    op=mybir.AluOpType.add)
            rec3 = statp.tile([128, 1], F32, tag="rc", name="rc3")
            nc.vector.reciprocal(rec3[:], ssum3[:])
            ot3 = obp.tile([128, S], F16, tag="ot", name="ot3")
            nc.vector.tensor_scalar_mul(ot3[:], exs3[:], rec3[:])
            nc.sync.dma_start(out=out[ts(3, 128), :], in_=ot3[:])

    nc.compile()
    return nc


_NC = None


def _get_nc():
    global _NC
    if _NC is None:
        _NC = build_nc()
    return _NC


def make_in_maps(query, key, mask, Wq, bq, Wk, bk, Wc, bc):
    query = np.asarray(query, np.float32)
    key = np.asarray(key, np.float32)
    mask = np.asarray(mask)
    Wq = np.asarray(Wq, np.float32)
    Wk = np.asarray(Wk, np.float32)
    Wc = np.asarray(Wc, np.float32)
    bq = np.asarray(bq, np.float32)
    bk = np.asarray(bk, np.float32)

    def blockdiag(W):
        blk = np.zeros((128, 128), np.float32)
        blk[0:64, 0:64] = W.T
        blk[64:128, 64:128] = W.T
        return blk

    wts = np.zeros((128, 384), np.float32)
    wts[:, 0:128] = blockdiag(Wk)
    wts[:, 128:256] = blockdiag(Wq)
    wts[:, 256:384] = np.eye(128, dtype=np.float32)
    wts = wts.astype(NP_BF16)

    aux = np.zeros((128, AFREE), np.float32)
    aux[:, AOFF_BK] = np.tile(bk.reshape(-1), 2)
    aux[:, AOFF_BQ] = np.tile(bq.reshape(-1), 2)
    for j in range(NJ):
        aux[0:64, AOFF_WC + j] = Wc[0, 2 * j]
        aux[64:128, AOFF_WC + j] = Wc[0, 2 * j + 1]

    in_maps = []
    for c in range(NCORES):
        b, half = divmod(c, 2)
        s0 = half * SQ
        qh = query[b].reshape(H, S, DK)[:, s0:s0 + SQ, :]
        qTc = np.ascontiguousarray(
            qh.transpose(0, 2, 1)).reshape(NJ, 128, SQ).astype(NP_BF16)
        kh_ = key[b].reshape(H, S, DK)
        kTc = np.ascontiguousarray(
            kh_.transpose(0, 2, 1)).reshape(NJ, 128, S).astype(NP_BF16)

        mc = np.ascontiguousarray(
            mask[b, s0:s0 + SQ, :].reshape(4, 128, S)).astype(np.int8)
        mka = np.concatenate([
            ((mask[b, s0 + 256:s0 + 384, 512:1024] == 0)
             .astype(np.float32) * -30000.0),
            ((mask[b, s0 + 384:s0 + 512, 512:1024] == 0)
             .astype(np.float32) * -30000.0)], axis=1).astype(NP_BF16)
        in_maps.append({"qT": qTc, "kT": kTc, "msk": mc, "wts": wts,
                        "aux": aux, "mska": mka})
    return in_maps


def kernel(query, key, mask, Wq, bq, Wk, bk, Wc, bc):
    from concourse.bass_utils import run_bass_kernel_spmd

    nc = _get_nc()
    in_maps = make_in_maps(query, key, mask, Wq, bq, Wk, bk, Wc, bc)
    res = run_bass_kernel_spmd(nc, in_maps, list(range(NCORES)))
    full = np.empty((B, S, S), np.float32)
    for c in range(NCORES):
        b, half = divmod(c, 2)
        full[b, half * SQ:(half + 1) * SQ, :] = \
            res.results[c]["out"].astype(np.float32)
    return full
